# revision 11
# baseline (speedup 1.0000x reference)
"""Trainium2 Bass kernel for nn_Downstream_38439957299924 (gnn_message_passing).

Distributed over 8 NeuronCores: 1D node partition (1024 rows/core).

Pipeline (reformulated, validated vs reference at ~9e-7 rel err in numpy):
  fea   = elu(x * wcomb)                          wcomb = cw00*prompt + cw01*shared
  agg   = Anorm @ fea                             Anorm built dense on host from edges
  h     = concat(fea, agg) * balance_tok ; hn = h / (||h|| + eps)
  sims  = hn @ hn.T ; t_i = 17th largest of row i
  Wsym  = relu(sims * (sims >= min(t_i, t_j)))    == to_undirected(mean)+relu of ref
  A_tot = alpha*Anorm + (1-alpha)*Wsym
  h1    = relu((A_tot @ fea) @ W1) ; emb = (A_tot @ h1) @ W2
  out   = cos(emb[node_idx], class prototypes) / TEMP

Sharding: each core owns N/8 node rows. fea/hnT/h1/emb are all-gathered; the
N x N stages only materialize [128, *] tiles; per-row top-k via max8 +
match_replace candidate extraction; Wsym needs no transpose communication
because sims is symmetric and the mask only needs the threshold vector t.
"""
import numpy as np

import concourse.bacc as bacc
import concourse.bass as bass
import concourse.mybir as mybir
import concourse.tile as tile
from concourse.bass_utils import run_bass_kernel_spmd
from concourse.masks import make_identity

FP = mybir.dt.float32
AF = mybir.ActivationFunctionType
ALU = mybir.AluOpType

NCORES = 8
N = 8192          # nodes
F = 256           # input feature dim
H2 = 512          # concat feature dim
HID = 256         # gnn hidden dim
NCLS = 10
NSEL = 4096
TEMP = 0.2
EPS = 1e-8
P = 128

NEGINF = -3.0e38


def build_program(alpha: float, n=N, ncores=NCORES, nsel=NSEL, mm_dtype=FP):
    """Emit the SPMD Bass program for one core (SPMD across ncores)."""
    R = n // ncores           # rows per core
    NB = n // P               # global node blocks of 128
    LB = R // P               # local node blocks
    KB = H2 // P              # feature k-blocks (4)
    FB = F // P               # 256-dim k-blocks (2)
    NCH = n // 512            # 512-wide j chunks globally
    NCH_L = R // 512          # 512-wide i chunks locally
    CH_PER_RANK = R // 512
    SELC = nsel // ncores     # selected nodes per core
    SB = SELC // P            # selected blocks per core
    QCH = (LB + 1) // 2       # psum tiles (2 x 256-wide outputs each)
    one_m_alpha = 1.0 - alpha
    # host ships alpha*AnormT when alpha is meaningful (lets phase C use a plain
    # tensor add on gpsimd); agg then un-scales at psum evict.
    use_anorm = alpha > 1e-6
    agg_scale = (1.0 / alpha) if use_anorm else 1.0
    assert R % 512 == 0 and SELC % P == 0

    def mmc(ap):
        """Bitcast a matmul operand to the matmul compute dtype (e.g. f32r)."""
        return ap.bitcast(mm_dtype) if mm_dtype != FP else ap

    nc = bacc.Bacc(None)

    # ---- per-core external inputs ----
    x_l = nc.declare_dram_parameter("x_l", [R, F], FP, isOutput=False)
    wcomb = nc.declare_dram_parameter("wcomb", [1, F], FP, isOutput=False)
    baltok = nc.declare_dram_parameter("baltok", [1, H2], FP, isOutput=False)
    anormT = nc.declare_dram_parameter("anormT", [n, R], FP, isOutput=False)
    w1 = nc.declare_dram_parameter("w1", [F, HID], FP, isOutput=False)
    w2 = nc.declare_dram_parameter("w2", [HID, HID], FP, isOutput=False)
    selidx = nc.declare_dram_parameter("selidx", [P, SB], mybir.dt.int32, isOutput=False)
    onehot = nc.declare_dram_parameter("onehot", [SELC, NCLS], FP, isOutput=False)
    out = nc.declare_dram_parameter("out", [SELC, NCLS], FP, isOutput=True)

    # ---- internal DRAM ----
    fea_bounce = nc.dram_tensor("fea_bounce", [R, F], FP)
    fea_full = nc.dram_tensor("fea_full", [n, F], FP, addr_space="Shared")
    hnT_bounce = nc.dram_tensor("hnT_bounce", [H2, R], FP)
    hnT_all = nc.dram_tensor("hnT_all", [ncores * H2, R], FP, addr_space="Shared")
    t_bounce = nc.dram_tensor("t_bounce", [R, 1], FP)
    t_full = nc.dram_tensor("t_full", [n, 1], FP, addr_space="Shared")
    atotT_dram = nc.dram_tensor("atotT_dram", [n, R], FP)
    h1_bounce = nc.dram_tensor("h1_bounce", [R, HID], FP)
    h1_full = nc.dram_tensor("h1_full", [n, HID], FP, addr_space="Shared")
    emb_bounce = nc.dram_tensor("emb_bounce", [R, HID], FP)
    emb_full = nc.dram_tensor("emb_full", [n, HID], FP, addr_space="Shared")
    sums_bounce = nc.dram_tensor("sums_bounce", [HID, NCLS], FP)
    sums_red = nc.dram_tensor("sums_red", [HID, NCLS], FP, addr_space="Shared")

    rg = [list(range(ncores))]

    def ag(in_ap, out_ap):
        nc.gpsimd.collective_compute(
            "AllGather", ALU.bypass, replica_groups=rg, ins=[in_ap], outs=[out_ap])

    def hnT_all_tile(jblk, kk):
        """[128 k, 128 j] slice of the gathered hnT for global node block jblk."""
        r, jb = jblk // LB, jblk % LB
        base = r * H2 + kk * P
        return hnT_all[base:base + P, jb * P:(jb + 1) * P]

    with tile.TileContext(nc) as tc:
        with (
            tc.tile_pool(name="const", bufs=1) as const,
            tc.tile_pool(name="live", bufs=1) as live,
        ):
            ident = const.tile([P, P], FP)
            make_identity(nc, ident)
            wcomb_b = const.tile([P, F], FP)
            nc.sync.dma_start(wcomb_b[:], wcomb[:1, :].to_broadcast([P, F]))
            baltok_b = const.tile([P, H2], FP)
            nc.sync.dma_start(baltok_b[:], baltok[:1, :].to_broadcast([P, H2]))
            hnT_l = [live.tile([P, R], FP, tag=f"hnT{k}", name=f"hnT{k}") for k in range(KB)]

            # ===== phase 0: fea_l = elu(x_l * wcomb); all-gather fea =====
            with tc.tile_pool(name="p0", bufs=3) as p0:
                for b in range(LB):
                    xt = p0.tile([P, F], FP, tag="xt")
                    nc.sync.dma_start(xt[:], x_l[b * P:(b + 1) * P, :])
                    z = p0.tile([P, F], FP, tag="z")
                    nc.vector.tensor_mul(z[:], xt[:], wcomb_b[:, :F])
                    r = p0.tile([P, F], FP, tag="r")
                    nc.scalar.activation(r[:], z[:], AF.Relu)
                    m0 = p0.tile([P, F], FP, tag="m0")
                    nc.vector.tensor_scalar_min(m0[:], z[:], 0.0)
                    e = p0.tile([P, F], FP, tag="e")
                    nc.scalar.activation(e[:], m0[:], AF.Exp)
                    ft = p0.tile([P, F], FP, tag="ft")
                    nc.vector.scalar_tensor_tensor(
                        ft[:], e[:], -1.0, r[:], op0=ALU.add, op1=ALU.add)
                    nc.sync.dma_start(fea_bounce[b * P:(b + 1) * P, :], ft[:])
            ag(fea_bounce[:], fea_full[:])

            # ===== phase A: aggT = fea.T @ anormT; then h, hn, hnT =====
            with tc.tile_pool(name="pa_sb", bufs=1) as pa_sb:
                aggT_sb = [pa_sb.tile([P, R], FP, tag=f"aggT{mf}", name=f"aggT{mf}") for mf in range(FB)]
                with (
                    tc.tile_pool(name="pa_in", bufs=3) as pa_in,
                    tc.tile_pool(name="pa_ps", bufs=1, space="PSUM") as pa_ps,
                ):
                    aggT_ps = [[pa_ps.tile([P, 512], FP, tag=f"aggps{mf}_{c2}", name=f"aggps{mf}_{c2}")
                                for c2 in range(NCH_L)] for mf in range(FB)]
                    for kj in range(NB):
                        fk = pa_in.tile([P, F], FP, tag="fk")
                        nc.sync.dma_start(fk[:], fea_full[kj * P:(kj + 1) * P, :])
                        an = pa_in.tile([P, R], FP, tag="an")
                        nc.sync.dma_start(an[:], anormT[kj * P:(kj + 1) * P, :])
                        for mf in range(FB):
                            for c2 in range(NCH_L):
                                nc.tensor.matmul(
                                    aggT_ps[mf][c2][:],
                                    mmc(fk[:, mf * P:(mf + 1) * P]),
                                    mmc(an[:, c2 * 512:(c2 + 1) * 512]),
                                    start=(kj == 0), stop=(kj == NB - 1))
                    for mf in range(FB):
                        for c2 in range(NCH_L):
                            nc.scalar.activation(
                                aggT_sb[mf][:, c2 * 512:(c2 + 1) * 512],
                                aggT_ps[mf][c2][:], AF.Copy, scale=agg_scale)

                with (
                    tc.tile_pool(name="ph_ps", bufs=2, space="PSUM") as ph_ps,
                    tc.tile_pool(name="ph_sb", bufs=2) as ph_sb,
                ):
                    for b in range(LB):
                        h = ph_sb.tile([P, H2], FP, tag="h")
                        ft = ph_sb.tile([P, F], FP, tag="ftl")
                        nc.sync.dma_start(ft[:], fea_bounce[b * P:(b + 1) * P, :])
                        nc.vector.tensor_mul(h[:, :F], ft[:], baltok_b[:, :F])
                        for mf in range(FB):
                            tp = ph_ps.tile([P, P], FP, tag="tp")
                            nc.tensor.transpose(
                                tp[:], aggT_sb[mf][:, b * P:(b + 1) * P], ident[:])
                            nc.scalar.activation(
                                h[:, F + mf * P:F + (mf + 1) * P], tp[:], AF.Copy)
                        nc.vector.tensor_mul(h[:, F:], h[:, F:], baltok_b[:, F:])
                        sq = ph_sb.tile([P, H2], FP, tag="sq")
                        ssq = ph_sb.tile([P, 1], FP, tag="ssq")
                        nc.scalar.activation(sq[:], h[:], AF.Square, accum_out=ssq[:])
                        nrm = ph_sb.tile([P, 1], FP, tag="nrm")
                        nc.scalar.activation(nrm[:], ssq[:], AF.Sqrt)
                        nc.vector.tensor_scalar_add(nrm[:], nrm[:], EPS)
                        inv = ph_sb.tile([P, 1], FP, tag="inv")
                        nc.vector.reciprocal(inv[:], nrm[:])
                        hn = ph_sb.tile([P, H2], FP, tag="hn")
                        nc.vector.tensor_scalar(hn[:], h[:], inv[:, :1], None, ALU.mult)
                        for kk in range(KB):
                            tp2 = ph_ps.tile([P, P], FP, tag="tp2")
                            nc.tensor.transpose(
                                tp2[:], hn[:, kk * P:(kk + 1) * P], ident[:])
                            nc.scalar.activation(
                                hnT_l[kk][:, b * P:(b + 1) * P], tp2[:], AF.Copy)
                    for kk in range(KB):
                        nc.sync.dma_start(
                            hnT_bounce[kk * P:(kk + 1) * P, :], hnT_l[kk][:])
            ag(hnT_bounce[:], hnT_all[:])

            # ===== phase B: per-row 17th-largest threshold t; all-gather t =====
            with (
                tc.tile_pool(name="pb_rhs", bufs=2) as pb_rhs,
                tc.tile_pool(name="pb_ps", bufs=3, space="PSUM") as pb_ps,
                tc.tile_pool(name="pb_sb", bufs=3) as pb_sb,
                tc.tile_pool(name="pb_cand", bufs=1) as pb_cand,
            ):
                cand = [pb_cand.tile([P, 16 * NCH], FP, tag=f"cand{m8}", name=f"cand{m8}")
                        for m8 in range(LB)]
                for nch in range(NCH):
                    rr, half = nch // CH_PER_RANK, nch % CH_PER_RANK
                    rhs = []
                    for kk in range(KB):
                        rt = pb_rhs.tile([P, 512], FP, tag=f"rhs{kk}")
                        base = rr * H2 + kk * P
                        nc.sync.dma_start(
                            rt[:],
                            hnT_all[base:base + P, half * 512:(half + 1) * 512])
                        rhs.append(rt)
                    for m8 in range(LB):
                        ps = pb_ps.tile([P, 512], FP, tag="ps")
                        for kk in range(KB):
                            nc.tensor.matmul(
                                ps[:], mmc(hnT_l[kk][:, m8 * P:(m8 + 1) * P]),
                                mmc(rhs[kk][:]),
                                start=(kk == 0), stop=(kk == KB - 1))
                        sb = pb_sb.tile([P, 512], FP, tag="sb")
                        nc.scalar.activation(sb[:], ps[:], AF.Copy)
                        nc.vector.max(cand[m8][:, nch * 16:nch * 16 + 8], sb[:])
                        nc.vector.match_replace(
                            sb[:], cand[m8][:, nch * 16:nch * 16 + 8], sb[:], NEGINF)
                        nc.vector.max(cand[m8][:, nch * 16 + 8:nch * 16 + 16], sb[:])
                for m8 in range(LB):
                    t8a = pb_sb.tile([P, 8], FP, tag="t8a")
                    nc.vector.max(t8a[:], cand[m8][:])
                    nc.vector.match_replace(cand[m8][:], t8a[:], cand[m8][:], NEGINF)
                    t8b = pb_sb.tile([P, 8], FP, tag="t8b")
                    nc.vector.max(t8b[:], cand[m8][:])
                    nc.vector.match_replace(cand[m8][:], t8b[:], cand[m8][:], NEGINF)
                    t8c = pb_sb.tile([P, 8], FP, tag="t8c")
                    nc.vector.max(t8c[:], cand[m8][:])
                    nc.sync.dma_start(t_bounce[m8 * P:(m8 + 1) * P, :], t8c[:, :1])
            ag(t_bounce[:], t_full[:])

            # ===== phase C: A_totT tiles + fused Q = A_tot @ fea =====
            with tc.tile_pool(name="pcd_sb", bufs=1) as pcd_sb:
                q_sb = [pcd_sb.tile([P, F], FP, tag=f"qsb{m8}", name=f"qsb{m8}") for m8 in range(LB)]
                with (
                    tc.tile_pool(name="pc_tib", bufs=1) as pc_tib,
                    tc.tile_pool(name="pc_in", bufs=3) as pc_in,
                    tc.tile_pool(name="pc_ps", bufs=2, space="PSUM") as pc_ps,
                    tc.tile_pool(name="pc_qps", bufs=1, space="PSUM") as pc_qps,
                    tc.tile_pool(name="pc_sb", bufs=3) as pc_sb,
                ):
                    t_i_b = pc_tib.tile([P, R], FP)
                    nc.sync.dma_start(
                        t_i_b[:],
                        t_bounce.rearrange("a b -> b a")[:1, :].to_broadcast([P, R]))
                    qps = [pc_qps.tile([P, 512], FP, tag=f"q{q}", name=f"q{q}") for q in range(QCH)]
                    for mj in range(NB):
                        lhs = []
                        for kk in range(KB):
                            lt = pc_in.tile([P, P], FP, tag=f"lhs{kk}")
                            nc.sync.dma_start(lt[:], hnT_all_tile(mj, kk))
                            lhs.append(lt)
                        tj = pc_in.tile([P, 1], FP, tag="tj")
                        nc.sync.dma_start(tj[:], t_full[mj * P:(mj + 1) * P, :])
                        feq = pc_in.tile([P, F], FP, tag="feq")
                        nc.sync.dma_start(feq[:], fea_full[mj * P:(mj + 1) * P, :])
                        tmin = pc_sb.tile([P, R], FP, tag="tmin")
                        nc.vector.tensor_scalar(
                            tmin[:], t_i_b[:], tj[:, :1], None, ALU.min)
                        for hh in range(NCH_L):
                            if use_anorm:
                                an = pc_in.tile([P, 512], FP, tag="an")
                                nc.sync.dma_start(
                                    an[:],
                                    anormT[mj * P:(mj + 1) * P,
                                           hh * 512:(hh + 1) * 512])
                            ps = pc_ps.tile([P, 512], FP, tag="ps")
                            for kk in range(KB):
                                nc.tensor.matmul(
                                    ps[:], mmc(lhs[kk][:]),
                                    mmc(hnT_l[kk][:, hh * 512:(hh + 1) * 512]),
                                    start=(kk == 0), stop=(kk == KB - 1))
                            ge = pc_sb.tile([P, 512], FP, tag="ge")
                            nc.vector.tensor_tensor(
                                ge[:], ps[:], tmin[:, hh * 512:(hh + 1) * 512],
                                ALU.is_ge)
                            w = pc_sb.tile([P, 512], FP, tag="w")
                            nc.vector.tensor_mul(w[:], ps[:], ge[:])
                            ws = pc_sb.tile([P, 512], FP, tag="ws")
                            nc.scalar.activation(ws[:], w[:], AF.Relu, scale=one_m_alpha)
                            at = pc_sb.tile([P, 512], FP, tag="at")
                            if use_anorm:
                                nc.gpsimd.tensor_tensor(at[:], an[:], ws[:], ALU.add)
                            else:
                                nc.vector.tensor_copy(at[:], ws[:])
                            nc.sync.dma_start(
                                atotT_dram[mj * P:(mj + 1) * P,
                                           hh * 512:(hh + 1) * 512], at[:])
                            for s in range(4):
                                m8 = hh * 4 + s
                                if m8 >= LB:
                                    continue
                                # start zeroes the whole 2KB psum region, so only
                                # the even half starts; only the odd half stops.
                                nc.tensor.matmul(
                                    qps[m8 // 2][:, (m8 % 2) * F:(m8 % 2 + 1) * F],
                                    mmc(at[:, s * P:(s + 1) * P]), mmc(feq[:]),
                                    start=(mj == 0 and m8 % 2 == 0),
                                    stop=(mj == NB - 1 and m8 % 2 == 1))
                    for m8 in range(LB):
                        nc.scalar.activation(
                            q_sb[m8][:],
                            qps[m8 // 2][:, (m8 % 2) * F:(m8 % 2 + 1) * F], AF.Copy)

                # ===== phase D1: h1 = relu(Q @ W1); all-gather h1 =====
                with (
                    tc.tile_pool(name="pd1_ps", bufs=2, space="PSUM") as pd1_ps,
                    tc.tile_pool(name="pd1_sb", bufs=2) as pd1_sb,
                    tc.tile_pool(name="pd1_w", bufs=1) as pd1_w,
                ):
                    w1_sb = [pd1_w.tile([P, HID], FP, tag=f"w1_{k2}", name=f"w1_{k2}")
                             for k2 in range(FB)]
                    for k2 in range(FB):
                        nc.sync.dma_start(w1_sb[k2][:], w1[k2 * P:(k2 + 1) * P, :])
                    qT = [pd1_w.tile([P, R], FP, tag=f"qT{k2}", name=f"qT{k2}")
                          for k2 in range(FB)]
                    for m8 in range(LB):
                        for k2 in range(FB):
                            tp = pd1_ps.tile([P, P], FP, tag="tp")
                            nc.tensor.transpose(
                                tp[:], q_sb[m8][:, k2 * P:(k2 + 1) * P], ident[:])
                            nc.scalar.activation(
                                qT[k2][:, m8 * P:(m8 + 1) * P], tp[:], AF.Copy)
                    for m8 in range(LB):
                        ps = pd1_ps.tile([P, HID], FP, tag="psh")
                        for k2 in range(FB):
                            nc.tensor.matmul(
                                ps[:], mmc(qT[k2][:, m8 * P:(m8 + 1) * P]),
                                mmc(w1_sb[k2][:]),
                                start=(k2 == 0), stop=(k2 == FB - 1))
                        h1t = pd1_sb.tile([P, HID], FP, tag="h1t")
                        nc.scalar.activation(h1t[:], ps[:], AF.Relu)
                        nc.sync.dma_start(h1_bounce[m8 * P:(m8 + 1) * P, :], h1t[:])
            ag(h1_bounce[:], h1_full[:])

            # ===== phase D2: P = A_tot @ h1_full =====
            with tc.tile_pool(name="pdd_sb", bufs=1) as pdd_sb:
                p_sb = [pdd_sb.tile([P, HID], FP, tag=f"psb{m8}", name=f"psb{m8}") for m8 in range(LB)]
                with (
                    tc.tile_pool(name="pd2_in", bufs=3) as pd2_in,
                    tc.tile_pool(name="pd2_ps", bufs=1, space="PSUM") as pd2_ps,
                ):
                    pps = [pd2_ps.tile([P, 512], FP, tag=f"p{q}", name=f"pp{q}") for q in range(QCH)]
                    for kj in range(NB):
                        at = pd2_in.tile([P, R], FP, tag="at")
                        nc.sync.dma_start(at[:], atotT_dram[kj * P:(kj + 1) * P, :])
                        h1k = pd2_in.tile([P, HID], FP, tag="h1k")
                        nc.sync.dma_start(h1k[:], h1_full[kj * P:(kj + 1) * P, :])
                        for m8 in range(LB):
                            nc.tensor.matmul(
                                pps[m8 // 2][:, (m8 % 2) * HID:(m8 % 2 + 1) * HID],
                                mmc(at[:, m8 * P:(m8 + 1) * P]), mmc(h1k[:]),
                                start=(kj == 0 and m8 % 2 == 0),
                                stop=(kj == NB - 1 and m8 % 2 == 1))
                    for m8 in range(LB):
                        nc.scalar.activation(
                            p_sb[m8][:],
                            pps[m8 // 2][:, (m8 % 2) * HID:(m8 % 2 + 1) * HID],
                            AF.Copy)

                # ===== phase D3: emb = P @ W2; all-gather emb =====
                with (
                    tc.tile_pool(name="pd3_ps", bufs=2, space="PSUM") as pd3_ps,
                    tc.tile_pool(name="pd3_w", bufs=1) as pd3_w,
                    tc.tile_pool(name="pd3_sb", bufs=2) as pd3_sb,
                ):
                    w2_sb = [pd3_w.tile([P, HID], FP, tag=f"w2_{k2}", name=f"w2_{k2}")
                             for k2 in range(FB)]
                    for k2 in range(FB):
                        nc.sync.dma_start(w2_sb[k2][:], w2[k2 * P:(k2 + 1) * P, :])
                    pT = [pd3_w.tile([P, R], FP, tag=f"pT{k2}", name=f"pT{k2}")
                          for k2 in range(FB)]
                    for m8 in range(LB):
                        for k2 in range(FB):
                            tp = pd3_ps.tile([P, P], FP, tag="tp")
                            nc.tensor.transpose(
                                tp[:], p_sb[m8][:, k2 * P:(k2 + 1) * P], ident[:])
                            nc.scalar.activation(
                                pT[k2][:, m8 * P:(m8 + 1) * P], tp[:], AF.Copy)
                    for m8 in range(LB):
                        ps = pd3_ps.tile([P, HID], FP, tag="pse")
                        for k2 in range(FB):
                            nc.tensor.matmul(
                                ps[:], mmc(pT[k2][:, m8 * P:(m8 + 1) * P]),
                                mmc(w2_sb[k2][:]),
                                start=(k2 == 0), stop=(k2 == FB - 1))
                        et = pd3_sb.tile([P, HID], FP, tag="et")
                        nc.scalar.activation(et[:], ps[:], AF.Copy)
                        nc.sync.dma_start(emb_bounce[m8 * P:(m8 + 1) * P, :], et[:])
            ag(emb_bounce[:], emb_full[:])

            # ===== phase E: prototypes + cosine scores =====
            with (
                tc.tile_pool(name="pe_sb", bufs=1) as pe_sb,
                tc.tile_pool(name="pe_ps", bufs=1, space="PSUM") as pe_ps,
                tc.tile_pool(name="pe_sc", bufs=2) as pe_sc,
            ):
                idx_sb = pe_sb.tile([P, SB], mybir.dt.int32)
                nc.sync.dma_start(idx_sb[:], selidx[:])
                sel_sb = [pe_sb.tile([P, HID], FP, tag=f"sel{q}", name=f"sel{q}") for q in range(SB)]
                sc_q = [pe_sb.tile([P, 1], FP, tag=f"scq{q}", name=f"scq{q}") for q in range(SB)]
                oh_sb = [pe_sb.tile([P, NCLS], FP, tag=f"oh{q}", name=f"oh{q}") for q in range(SB)]
                for q in range(SB):
                    nc.gpsimd.indirect_dma_start(
                        out=sel_sb[q][:], out_offset=None,
                        in_=emb_full[:],
                        in_offset=bass.IndirectOffsetOnAxis(
                            ap=idx_sb[:, q:q + 1], axis=0))
                    nc.sync.dma_start(oh_sb[q][:], onehot[q * P:(q + 1) * P, :])
                    sq = pe_sc.tile([P, HID], FP, tag="sq")
                    ssq = pe_sc.tile([P, 1], FP, tag="ssq")
                    nc.scalar.activation(
                        sq[:], sel_sb[q][:], AF.Square, accum_out=ssq[:])
                    nrm = pe_sc.tile([P, 1], FP, tag="nrm2")
                    nc.scalar.activation(nrm[:], ssq[:], AF.Sqrt)
                    nc.vector.tensor_scalar_add(nrm[:], nrm[:], EPS)
                    nc.vector.tensor_scalar_mul(nrm[:], nrm[:], TEMP)
                    nc.vector.reciprocal(sc_q[q][:], nrm[:])
                sums_ps = [pe_ps.tile([P, NCLS], FP, tag=f"sums{b2}", name=f"sums{b2}")
                           for b2 in range(FB)]
                for q in range(SB):
                    for b2 in range(FB):
                        nc.tensor.matmul(
                            sums_ps[b2][:], sel_sb[q][:, b2 * P:(b2 + 1) * P],
                            oh_sb[q][:], start=(q == 0), stop=(q == SB - 1))
                for b2 in range(FB):
                    st = pe_sc.tile([P, NCLS], FP, tag="st")
                    nc.scalar.activation(st[:], sums_ps[b2][:], AF.Copy)
                    nc.sync.dma_start(sums_bounce[b2 * P:(b2 + 1) * P, :], st[:])
                nc.gpsimd.collective_compute(
                    "AllReduce", ALU.add, replica_groups=rg,
                    ins=[sums_bounce[:]], outs=[sums_red[:]])
                sums_sb = [pe_sb.tile([P, NCLS], FP, tag=f"smr{b2}", name=f"smr{b2}")
                           for b2 in range(FB)]
                ones_col = pe_sb.tile([P, 1], FP)
                nc.vector.memset(ones_col[:], 1.0)
                ones_row = pe_sb.tile([1, P], FP)
                nc.vector.memset(ones_row[:1, :], 1.0)
                nps = pe_ps.tile([1, NCLS], FP, tag="nps")
                for b2 in range(FB):
                    nc.sync.dma_start(sums_sb[b2][:], sums_red[b2 * P:(b2 + 1) * P, :])
                    sqs = pe_sc.tile([P, NCLS], FP, tag="sqs")
                    nc.scalar.activation(sqs[:], sums_sb[b2][:], AF.Square)
                    nc.tensor.matmul(nps[:1, :], ones_col[:, :1], sqs[:],
                                     start=(b2 == 0), stop=(b2 == FB - 1))
                nrmc = pe_sc.tile([1, NCLS], FP, tag="nrmc")
                nc.scalar.activation(nrmc[:1, :], nps[:1, :], AF.Sqrt)
                nc.vector.tensor_scalar_add(nrmc[:1, :], nrmc[:1, :], EPS)
                invc = pe_sc.tile([1, NCLS], FP, tag="invc")
                nc.vector.reciprocal(invc[:1, :], nrmc[:1, :])
                bcp = pe_ps.tile([P, NCLS], FP, tag="bcp")
                nc.tensor.matmul(bcp[:], ones_row[:1, :], invc[:1, :],
                                 start=True, stop=True)
                bc_sb = pe_sb.tile([P, NCLS], FP)
                nc.scalar.activation(bc_sb[:], bcp[:], AF.Copy)
                pnT = [pe_sb.tile([P, NCLS], FP, tag=f"pnT{b2}", name=f"pnT{b2}") for b2 in range(FB)]
                for b2 in range(FB):
                    nc.vector.tensor_mul(pnT[b2][:], sums_sb[b2][:], bc_sb[:])
                selT = [pe_sb.tile([P, SELC], FP, tag=f"selT{b2}", name=f"selT{b2}")
                        for b2 in range(FB)]
                for q in range(SB):
                    for b2 in range(FB):
                        tp = pe_ps.tile([P, P], FP, tag="tpe")
                        nc.tensor.transpose(
                            tp[:], sel_sb[q][:, b2 * P:(b2 + 1) * P], ident[:])
                        nc.scalar.activation(
                            selT[b2][:, q * P:(q + 1) * P], tp[:], AF.Copy)
                for q in range(SB):
                    ops = pe_ps.tile([P, NCLS], FP, tag="ops")
                    for b2 in range(FB):
                        nc.tensor.matmul(
                            ops[:], selT[b2][:, q * P:(q + 1) * P], pnT[b2][:],
                            start=(b2 == 0), stop=(b2 == FB - 1))
                    ot = pe_sc.tile([P, NCLS], FP, tag="ot")
                    nc.scalar.activation(ot[:], ops[:], AF.Copy, scale=sc_q[q][:, :1])
                    nc.sync.dma_start(out[q * P:(q + 1) * P, :], ot[:])

    nc.finalize()
    return nc


# ---------------------------------------------------------------------------
# host side
# ---------------------------------------------------------------------------

def host_preprocess(inputs, n=N, ncores=NCORES, nsel=NSEL):
    R = n // ncores
    selc = nsel // ncores
    x = np.ascontiguousarray(np.asarray(inputs["x"], dtype=np.float32))
    cw = np.asarray(inputs["combine_weight"], dtype=np.float32)
    alpha = float(np.asarray(inputs["alpha"], dtype=np.float32))
    prompt = np.asarray(inputs["prompt_spec"], dtype=np.float32)
    shared = np.asarray(inputs["shared_tok"], dtype=np.float32)
    baltok = np.asarray(inputs["balance_tok"], dtype=np.float32)
    w1 = np.ascontiguousarray(np.asarray(inputs["W1"], dtype=np.float32))
    w2 = np.ascontiguousarray(np.asarray(inputs["W2"], dtype=np.float32))
    edge_index = np.asarray(inputs["edge_index"])
    labels = np.asarray(inputs["labels"])
    node_idx = np.asarray(inputs["node_idx"])

    src = edge_index[0].astype(np.int64)
    dst = edge_index[1].astype(np.int64)
    deg = (np.bincount(dst, minlength=n) + 1).astype(np.float32)
    dinv = deg ** -0.5
    wn = (dinv[src] * dinv[dst]).astype(np.float32)
    # AnormT[src, dst] += wn  (transpose of reference's Anorm[dst, src] += wn)
    anormT = np.zeros((n, n), dtype=np.float32)
    np.add.at(anormT, (src, dst), wn)
    anormT[np.arange(n), np.arange(n)] += dinv * dinv
    if alpha > 1e-6:
        # device expects alpha-scaled adjacency (see build_program use_anorm)
        anormT *= alpha

    wcomb = (cw[0, 0] * prompt + cw[0, 1] * shared).astype(np.float32).reshape(1, -1)
    baltok2 = np.ascontiguousarray(baltok.reshape(1, -1))

    nsel_here = node_idx.shape[0]
    onehot_all = np.zeros((nsel_here, NCLS), dtype=np.float32)
    onehot_all[np.arange(nsel_here), labels] = 1.0

    in_maps = []
    for c in range(ncores):
        sel_slice = node_idx[c * selc:(c + 1) * selc].astype(np.int32)
        sb = selc // P
        in_maps.append({
            "x_l": x[c * R:(c + 1) * R, :],
            "wcomb": wcomb,
            "baltok": baltok2,
            "anormT": np.ascontiguousarray(anormT[:, c * R:(c + 1) * R]),
            "w1": w1,
            "w2": w2,
            "selidx": np.ascontiguousarray(sel_slice.reshape(sb, P).T),
            "onehot": np.ascontiguousarray(onehot_all[c * selc:(c + 1) * selc, :]),
        })
    return alpha, in_maps


_prog_cache = {}


def kernel(**inputs) -> np.ndarray:
    alpha, in_maps = host_preprocess(inputs)
    key = round(alpha, 9)
    if key not in _prog_cache:
        _prog_cache[key] = build_program(alpha)
    nc = _prog_cache[key]
    res = run_bass_kernel_spmd(nc, in_maps, list(range(NCORES)))
    return np.concatenate([res.results[c]["out"] for c in range(NCORES)], axis=0)


# revision 21
# speedup vs baseline: 1.1434x; 1.1434x over previous
"""Trainium2 Bass kernel for nn_Downstream_38439957299924 (gnn_message_passing).

Distributed over 8 NeuronCores: 1D node partition (1024 rows/core).

Pipeline (reformulated, validated vs reference at ~9e-7 rel err in numpy):
  fea   = elu(x * wcomb)                          wcomb = cw00*prompt + cw01*shared
  agg   = Anorm @ fea                             Anorm built dense on host from edges
  h     = concat(fea, agg) * balance_tok ; hn = h / (||h|| + eps)
  sims  = hn @ hn.T ; t_i = 17th largest of row i
  Wsym  = relu(sims * (sims >= min(t_i, t_j)))    == to_undirected(mean)+relu of ref
  A_tot = alpha*Anorm + (1-alpha)*Wsym
  h1    = relu((A_tot @ fea) @ W1) ; emb = (A_tot @ h1) @ W2
  out   = cos(emb[node_idx], class prototypes) / TEMP

Sharding: each core owns N/8 node rows. fea/hnT/h1/emb are all-gathered; the
N x N stages only materialize [128, *] tiles; per-row top-k via max8 +
match_replace candidate extraction; Wsym needs no transpose communication
because sims is symmetric and the mask only needs the threshold vector t.
"""
import numpy as np

import concourse.bacc as bacc
import concourse.bass as bass
import concourse.mybir as mybir
import concourse.tile as tile
from concourse.bass_utils import run_bass_kernel_spmd
from concourse.masks import make_identity

FP = mybir.dt.float32
AF = mybir.ActivationFunctionType
ALU = mybir.AluOpType

NCORES = 8
N = 8192          # nodes
F = 256           # input feature dim
H2 = 512          # concat feature dim
HID = 256         # gnn hidden dim
NCLS = 10
NSEL = 4096
TEMP = 0.2
EPS = 1e-8
P = 128

NEGINF = -3.0e38


def build_program(alpha: float, n=N, ncores=NCORES, nsel=NSEL, mm_dtype=FP,
                  debug_outputs=False):
    """Emit the SPMD Bass program for one core (SPMD across ncores)."""
    R = n // ncores           # rows per core
    NB = n // P               # global node blocks of 128
    LB = R // P               # local node blocks
    KB = H2 // P              # feature k-blocks (4)
    FB = F // P               # 256-dim k-blocks (2)
    NCH = n // 512            # 512-wide j chunks globally
    NCH_L = R // 512          # 512-wide i chunks locally
    CH_PER_RANK = R // 512
    SELC = nsel // ncores     # selected nodes per core
    SB = SELC // P            # selected blocks per core
    QCH = (LB + 1) // 2       # psum tiles (2 x 256-wide outputs each)
    one_m_alpha = 1.0 - alpha
    # host ships alpha*AnormT when alpha is meaningful (lets phase C use a plain
    # tensor add on gpsimd); agg then un-scales at psum evict.
    use_anorm = alpha > 1e-6
    agg_scale = (1.0 / alpha) if use_anorm else 1.0
    assert R % 512 == 0 and SELC % P == 0

    nc = bacc.Bacc(None)

    # ---- per-core external inputs ----
    x_l = nc.declare_dram_parameter("x_l", [R, F], FP, isOutput=False)
    wcomb = nc.declare_dram_parameter("wcomb", [1, F], FP, isOutput=False)
    baltok = nc.declare_dram_parameter("baltok", [1, H2], FP, isOutput=False)
    anormT = nc.declare_dram_parameter("anormT", [n, R], FP, isOutput=False)
    w1 = nc.declare_dram_parameter("w1", [F, HID], FP, isOutput=False)
    w2 = nc.declare_dram_parameter("w2", [HID, HID], FP, isOutput=False)
    selidx = nc.declare_dram_parameter("selidx", [P, SB], mybir.dt.int32, isOutput=False)
    onehot = nc.declare_dram_parameter("onehot", [SELC, NCLS], FP, isOutput=False)
    out = nc.declare_dram_parameter("out", [SELC, NCLS], FP, isOutput=True)
    if debug_outputs:
        dbg_t = nc.declare_dram_parameter("dbg_t", [R, 1], FP, isOutput=True)
        dbg_hnT = nc.declare_dram_parameter("dbg_hnT", [H2, R], FP, isOutput=True)
        dbg_h1 = nc.declare_dram_parameter("dbg_h1", [R, HID], FP, isOutput=True)
        dbg_emb = nc.declare_dram_parameter("dbg_emb", [R, HID], FP, isOutput=True)
        dbg_agg = nc.declare_dram_parameter("dbg_agg", [F, R], FP, isOutput=True)

    # ---- internal DRAM ----
    fea_bounce = nc.dram_tensor("fea_bounce", [R, F], FP)
    fea_full = nc.dram_tensor("fea_full", [n, F], FP, addr_space="Shared")
    hnT_bounce = nc.dram_tensor("hnT_bounce", [H2, R], FP)
    hnT_all = nc.dram_tensor("hnT_all", [ncores * H2, R], FP, addr_space="Shared")
    t_bounce = nc.dram_tensor("t_bounce", [R, 1], FP)
    t_full = nc.dram_tensor("t_full", [n, 1], FP, addr_space="Shared")
    atotT_dram = nc.dram_tensor("atotT_dram", [n, R], mm_dtype)
    h1_bounce = nc.dram_tensor("h1_bounce", [R, HID], FP)
    h1_full = nc.dram_tensor("h1_full", [n, HID], FP, addr_space="Shared")
    emb_bounce = nc.dram_tensor("emb_bounce", [R, HID], FP)
    emb_full = nc.dram_tensor("emb_full", [n, HID], FP, addr_space="Shared")
    sums_bounce = nc.dram_tensor("sums_bounce", [HID, NCLS], FP)
    sums_red = nc.dram_tensor("sums_red", [HID, NCLS], FP, addr_space="Shared")

    rg = [list(range(ncores))]

    def dma_cast(out_ap, in_ap):
        """DMA that may convert f32 -> f32r; casting DMAs must come from gpsimd."""
        eng = nc.gpsimd if mm_dtype != FP else nc.sync
        eng.dma_start(out_ap, in_ap)

    def ag(in_ap, out_ap):
        nc.gpsimd.collective_compute(
            "AllGather", ALU.bypass, replica_groups=rg, ins=[in_ap], outs=[out_ap])

    def hnT_all_tile(jblk, kk):
        """[128 k, 128 j] slice of the gathered hnT for global node block jblk."""
        r, jb = jblk // LB, jblk % LB
        base = r * H2 + kk * P
        return hnT_all[base:base + P, jb * P:(jb + 1) * P]

    with tile.TileContext(nc) as tc:
        with (
            tc.tile_pool(name="const", bufs=1) as const,
            tc.tile_pool(name="live", bufs=1) as live,
        ):
            ident = const.tile([P, P], FP)
            make_identity(nc, ident)
            wcomb_b = const.tile([P, F], FP)
            nc.sync.dma_start(wcomb_b[:], wcomb[:1, :].to_broadcast([P, F]))
            baltok_b = const.tile([P, H2], FP)
            nc.sync.dma_start(baltok_b[:], baltok[:1, :].to_broadcast([P, H2]))
            hnT_l = [live.tile([P, R], FP, tag=f"hnT{k}", name=f"hnT{k}") for k in range(KB)]

            # ===== phase 0: fea_l = elu(x_l * wcomb); all-gather fea =====
            with tc.tile_pool(name="p0", bufs=3) as p0:
                for b in range(LB):
                    xt = p0.tile([P, F], FP, tag="xt")
                    nc.sync.dma_start(xt[:], x_l[b * P:(b + 1) * P, :])
                    z = p0.tile([P, F], FP, tag="z")
                    nc.vector.tensor_mul(z[:], xt[:], wcomb_b[:, :F])
                    r = p0.tile([P, F], FP, tag="r")
                    nc.scalar.activation(r[:], z[:], AF.Relu)
                    m0 = p0.tile([P, F], FP, tag="m0")
                    nc.vector.tensor_scalar_min(m0[:], z[:], 0.0)
                    e = p0.tile([P, F], FP, tag="e")
                    nc.scalar.activation(e[:], m0[:], AF.Exp)
                    ft = p0.tile([P, F], FP, tag="ft")
                    nc.vector.scalar_tensor_tensor(
                        ft[:], e[:], -1.0, r[:], op0=ALU.add, op1=ALU.add)
                    nc.sync.dma_start(fea_bounce[b * P:(b + 1) * P, :], ft[:])
            ag(fea_bounce[:], fea_full[:])

            # ===== phase A: aggT = fea.T @ anormT; then h, hn, hnT =====
            with tc.tile_pool(name="pa_sb", bufs=1) as pa_sb:
                aggT_sb = [pa_sb.tile([P, R], FP, tag=f"aggT{mf}", name=f"aggT{mf}") for mf in range(FB)]
                with (
                    tc.tile_pool(name="pa_in", bufs=3) as pa_in,
                    tc.tile_pool(name="pa_ps", bufs=1, space="PSUM") as pa_ps,
                ):
                    aggT_ps = [[pa_ps.tile([P, 512], FP, tag=f"aggps{mf}_{c2}", name=f"aggps{mf}_{c2}")
                                for c2 in range(NCH_L)] for mf in range(FB)]
                    for kj in range(NB):
                        fk = pa_in.tile([P, F], mm_dtype, tag="fk")
                        dma_cast(fk[:], fea_full[kj * P:(kj + 1) * P, :])
                        an = pa_in.tile([P, R], mm_dtype, tag="an")
                        dma_cast(an[:], anormT[kj * P:(kj + 1) * P, :])
                        for mf in range(FB):
                            for c2 in range(NCH_L):
                                nc.tensor.matmul(
                                    aggT_ps[mf][c2][:],
                                    fk[:, mf * P:(mf + 1) * P],
                                    an[:, c2 * 512:(c2 + 1) * 512],
                                    start=(kj == 0), stop=(kj == NB - 1))
                    for mf in range(FB):
                        for c2 in range(NCH_L):
                            nc.scalar.activation(
                                aggT_sb[mf][:, c2 * 512:(c2 + 1) * 512],
                                aggT_ps[mf][c2][:], AF.Copy, scale=agg_scale)
                    if debug_outputs:
                        for mf in range(FB):
                            nc.sync.dma_start(
                                dbg_agg[mf * P:(mf + 1) * P, :], aggT_sb[mf][:])

                with (
                    tc.tile_pool(name="ph_ps", bufs=2, space="PSUM") as ph_ps,
                    tc.tile_pool(name="ph_sb", bufs=2) as ph_sb,
                ):
                    for b in range(LB):
                        h = ph_sb.tile([P, H2], FP, tag="h")
                        ft = ph_sb.tile([P, F], FP, tag="ftl")
                        nc.sync.dma_start(ft[:], fea_bounce[b * P:(b + 1) * P, :])
                        nc.vector.tensor_mul(h[:, :F], ft[:], baltok_b[:, :F])
                        for mf in range(FB):
                            tp = ph_ps.tile([P, P], FP, tag="tp")
                            nc.tensor.transpose(
                                tp[:], aggT_sb[mf][:, b * P:(b + 1) * P], ident[:])
                            nc.scalar.activation(
                                h[:, F + mf * P:F + (mf + 1) * P], tp[:], AF.Copy)
                        nc.vector.tensor_mul(h[:, F:], h[:, F:], baltok_b[:, F:])
                        sq = ph_sb.tile([P, H2], FP, tag="sq")
                        ssq = ph_sb.tile([P, 1], FP, tag="ssq")
                        nc.scalar.activation(sq[:], h[:], AF.Square, accum_out=ssq[:])
                        nrm = ph_sb.tile([P, 1], FP, tag="nrm")
                        nc.scalar.activation(nrm[:], ssq[:], AF.Sqrt)
                        nc.vector.tensor_scalar_add(nrm[:], nrm[:], EPS)
                        inv = ph_sb.tile([P, 1], FP, tag="inv")
                        nc.vector.reciprocal(inv[:], nrm[:])
                        hn = ph_sb.tile([P, H2], FP, tag="hn")
                        nc.vector.tensor_scalar(hn[:], h[:], inv[:, :1], None, ALU.mult)
                        for kk in range(KB):
                            tp2 = ph_ps.tile([P, P], FP, tag="tp2")
                            nc.tensor.transpose(
                                tp2[:], hn[:, kk * P:(kk + 1) * P], ident[:])
                            nc.scalar.activation(
                                hnT_l[kk][:, b * P:(b + 1) * P], tp2[:], AF.Copy)
                    for kk in range(KB):
                        nc.sync.dma_start(
                            hnT_bounce[kk * P:(kk + 1) * P, :], hnT_l[kk][:])
            ag(hnT_bounce[:], hnT_all[:])

            # ===== phase B: per-row 17th-largest threshold t; all-gather t =====
            with (
                tc.tile_pool(name="pb_rhs", bufs=2) as pb_rhs,
                tc.tile_pool(name="pb_ps", bufs=3, space="PSUM") as pb_ps,
                tc.tile_pool(name="pb_sb", bufs=3) as pb_sb,
                tc.tile_pool(name="pb_cand", bufs=1) as pb_cand,
            ):
                cand = [pb_cand.tile([P, 16 * NCH], FP, tag=f"cand{m8}", name=f"cand{m8}")
                        for m8 in range(LB)]
                for nch in range(NCH):
                    rr, half = nch // CH_PER_RANK, nch % CH_PER_RANK
                    rhs = []
                    for kk in range(KB):
                        rt = pb_rhs.tile([P, 512], FP, tag=f"rhs{kk}")
                        base = rr * H2 + kk * P
                        nc.sync.dma_start(
                            rt[:],
                            hnT_all[base:base + P, half * 512:(half + 1) * 512])
                        rhs.append(rt)
                    for m8 in range(LB):
                        ps = pb_ps.tile([P, 512], FP, tag="ps")
                        for kk in range(KB):
                            nc.tensor.matmul(
                                ps[:], hnT_l[kk][:, m8 * P:(m8 + 1) * P],
                                rhs[kk][:],
                                start=(kk == 0), stop=(kk == KB - 1))
                        sb = pb_sb.tile([P, 512], FP, tag="sb")
                        nc.scalar.activation(sb[:], ps[:], AF.Copy)
                        nc.vector.max(cand[m8][:, nch * 16:nch * 16 + 8], sb[:])
                        nc.vector.match_replace(
                            sb[:], cand[m8][:, nch * 16:nch * 16 + 8], sb[:], NEGINF)
                        nc.vector.max(cand[m8][:, nch * 16 + 8:nch * 16 + 16], sb[:])
                for m8 in range(LB):
                    t8a = pb_sb.tile([P, 8], FP, tag="t8a")
                    nc.vector.max(t8a[:], cand[m8][:])
                    nc.vector.match_replace(cand[m8][:], t8a[:], cand[m8][:], NEGINF)
                    t8b = pb_sb.tile([P, 8], FP, tag="t8b")
                    nc.vector.max(t8b[:], cand[m8][:])
                    nc.vector.match_replace(cand[m8][:], t8b[:], cand[m8][:], NEGINF)
                    t8c = pb_sb.tile([P, 8], FP, tag="t8c")
                    nc.vector.max(t8c[:], cand[m8][:])
                    nc.sync.dma_start(t_bounce[m8 * P:(m8 + 1) * P, :], t8c[:, :1])
            ag(t_bounce[:], t_full[:])

            # ===== phase C: A_totT tiles + fused Q = A_tot @ fea =====
            with tc.tile_pool(name="pcd_sb", bufs=1) as pcd_sb:
                q_sb = [pcd_sb.tile([P, F], FP, tag=f"qsb{m8}", name=f"qsb{m8}") for m8 in range(LB)]
                with (
                    tc.tile_pool(name="pc_tib", bufs=1) as pc_tib,
                    tc.tile_pool(name="pc_in", bufs=3) as pc_in,
                    tc.tile_pool(name="pc_ps", bufs=2, space="PSUM") as pc_ps,
                    tc.tile_pool(name="pc_qps", bufs=1, space="PSUM") as pc_qps,
                    tc.tile_pool(name="pc_sb", bufs=3) as pc_sb,
                ):
                    t_i_b = pc_tib.tile([P, R], FP)
                    nc.sync.dma_start(
                        t_i_b[:],
                        t_bounce.rearrange("a b -> b a")[:1, :].to_broadcast([P, R]))
                    # t_full loaded once as [128, NB]: column mj = t for node block mj
                    tf_sb = pc_tib.tile([P, NB], FP)
                    nc.sync.dma_start(
                        tf_sb[:], t_full.rearrange("(m p) one -> p (m one)", p=P))
                    qps = [pc_qps.tile([P, 512], FP, tag=f"q{q}", name=f"q{q}") for q in range(QCH)]
                    for mj in range(NB):
                        lhs = []
                        for kk in range(KB):
                            lt = pc_in.tile([P, P], FP, tag=f"lhs{kk}")
                            nc.sync.dma_start(lt[:], hnT_all_tile(mj, kk))
                            lhs.append(lt)
                        feq = pc_in.tile([P, F], mm_dtype, tag="feq")
                        dma_cast(feq[:], fea_full[mj * P:(mj + 1) * P, :])
                        tmin = pc_sb.tile([P, R], FP, tag="tmin")
                        nc.vector.tensor_scalar(
                            tmin[:], t_i_b[:], tf_sb[:, mj:mj + 1], None, ALU.min)
                        for hh in range(NCH_L):
                            if use_anorm:
                                an = pc_in.tile([P, 512], FP, tag="an")
                                nc.sync.dma_start(
                                    an[:],
                                    anormT[mj * P:(mj + 1) * P,
                                           hh * 512:(hh + 1) * 512])
                            ps = pc_ps.tile([P, 512], FP, tag="ps")
                            for kk in range(KB):
                                nc.tensor.matmul(
                                    ps[:], lhs[kk][:],
                                    hnT_l[kk][:, hh * 512:(hh + 1) * 512],
                                    start=(kk == 0), stop=(kk == KB - 1))
                            ge = pc_sb.tile([P, 512], FP, tag="ge")
                            nc.vector.tensor_tensor(
                                ge[:], ps[:], tmin[:, hh * 512:(hh + 1) * 512],
                                ALU.is_ge)
                            w = pc_sb.tile([P, 512], FP, tag="w")
                            nc.vector.tensor_mul(w[:], ps[:], ge[:])
                            ws = pc_sb.tile([P, 512], FP, tag="ws")
                            nc.scalar.activation(ws[:], w[:], AF.Relu, scale=one_m_alpha)
                            at = pc_sb.tile([P, 512], mm_dtype, tag="at")
                            if use_anorm:
                                nc.gpsimd.tensor_tensor(at[:], an[:], ws[:], ALU.add)
                            else:
                                nc.vector.tensor_copy(at[:], ws[:])
                            nc.sync.dma_start(
                                atotT_dram[mj * P:(mj + 1) * P,
                                           hh * 512:(hh + 1) * 512], at[:])
                            for s in range(4):
                                m8 = hh * 4 + s
                                if m8 >= LB:
                                    continue
                                # start zeroes the whole 2KB psum region, so only
                                # the even half starts; only the odd half stops.
                                nc.tensor.matmul(
                                    qps[m8 // 2][:, (m8 % 2) * F:(m8 % 2 + 1) * F],
                                    at[:, s * P:(s + 1) * P], feq[:],
                                    start=(mj == 0 and m8 % 2 == 0),
                                    stop=(mj == NB - 1 and m8 % 2 == 1))
                    for m8 in range(LB):
                        nc.scalar.activation(
                            q_sb[m8][:],
                            qps[m8 // 2][:, (m8 % 2) * F:(m8 % 2 + 1) * F], AF.Copy)

                # ===== phase D1: h1 = relu(Q @ W1); all-gather h1 =====
                with (
                    tc.tile_pool(name="pd1_ps", bufs=2, space="PSUM") as pd1_ps,
                    tc.tile_pool(name="pd1_sb", bufs=2) as pd1_sb,
                    tc.tile_pool(name="pd1_w", bufs=1) as pd1_w,
                ):
                    w1_sb = [pd1_w.tile([P, HID], FP, tag=f"w1_{k2}", name=f"w1_{k2}")
                             for k2 in range(FB)]
                    for k2 in range(FB):
                        nc.sync.dma_start(w1_sb[k2][:], w1[k2 * P:(k2 + 1) * P, :])
                    qT = [pd1_w.tile([P, R], FP, tag=f"qT{k2}", name=f"qT{k2}")
                          for k2 in range(FB)]
                    for m8 in range(LB):
                        for k2 in range(FB):
                            tp = pd1_ps.tile([P, P], FP, tag="tp")
                            nc.tensor.transpose(
                                tp[:], q_sb[m8][:, k2 * P:(k2 + 1) * P], ident[:])
                            nc.scalar.activation(
                                qT[k2][:, m8 * P:(m8 + 1) * P], tp[:], AF.Copy)
                    for m8 in range(LB):
                        ps = pd1_ps.tile([P, HID], FP, tag="psh")
                        for k2 in range(FB):
                            nc.tensor.matmul(
                                ps[:], qT[k2][:, m8 * P:(m8 + 1) * P], w1_sb[k2][:],
                                start=(k2 == 0), stop=(k2 == FB - 1))
                        h1t = pd1_sb.tile([P, HID], FP, tag="h1t")
                        nc.scalar.activation(h1t[:], ps[:], AF.Relu)
                        nc.sync.dma_start(h1_bounce[m8 * P:(m8 + 1) * P, :], h1t[:])
            ag(h1_bounce[:], h1_full[:])

            # ===== phase D2: P = A_tot @ h1_full =====
            with tc.tile_pool(name="pdd_sb", bufs=1) as pdd_sb:
                p_sb = [pdd_sb.tile([P, HID], FP, tag=f"psb{m8}", name=f"psb{m8}") for m8 in range(LB)]
                with (
                    tc.tile_pool(name="pd2_in", bufs=3) as pd2_in,
                    tc.tile_pool(name="pd2_ps", bufs=1, space="PSUM") as pd2_ps,
                ):
                    pps = [pd2_ps.tile([P, 512], FP, tag=f"p{q}", name=f"pp{q}") for q in range(QCH)]
                    for kj in range(NB):
                        at = pd2_in.tile([P, R], mm_dtype, tag="at")
                        nc.sync.dma_start(at[:], atotT_dram[kj * P:(kj + 1) * P, :])
                        h1k = pd2_in.tile([P, HID], mm_dtype, tag="h1k")
                        dma_cast(h1k[:], h1_full[kj * P:(kj + 1) * P, :])
                        for m8 in range(LB):
                            nc.tensor.matmul(
                                pps[m8 // 2][:, (m8 % 2) * HID:(m8 % 2 + 1) * HID],
                                at[:, m8 * P:(m8 + 1) * P], h1k[:],
                                start=(kj == 0 and m8 % 2 == 0),
                                stop=(kj == NB - 1 and m8 % 2 == 1))
                    for m8 in range(LB):
                        nc.scalar.activation(
                            p_sb[m8][:],
                            pps[m8 // 2][:, (m8 % 2) * HID:(m8 % 2 + 1) * HID],
                            AF.Copy)

                # ===== phase D3: emb = P @ W2; all-gather emb =====
                with (
                    tc.tile_pool(name="pd3_ps", bufs=2, space="PSUM") as pd3_ps,
                    tc.tile_pool(name="pd3_w", bufs=1) as pd3_w,
                    tc.tile_pool(name="pd3_sb", bufs=2) as pd3_sb,
                ):
                    w2_sb = [pd3_w.tile([P, HID], FP, tag=f"w2_{k2}", name=f"w2_{k2}")
                             for k2 in range(FB)]
                    for k2 in range(FB):
                        nc.sync.dma_start(w2_sb[k2][:], w2[k2 * P:(k2 + 1) * P, :])
                    pT = [pd3_w.tile([P, R], FP, tag=f"pT{k2}", name=f"pT{k2}")
                          for k2 in range(FB)]
                    for m8 in range(LB):
                        for k2 in range(FB):
                            tp = pd3_ps.tile([P, P], FP, tag="tp")
                            nc.tensor.transpose(
                                tp[:], p_sb[m8][:, k2 * P:(k2 + 1) * P], ident[:])
                            nc.scalar.activation(
                                pT[k2][:, m8 * P:(m8 + 1) * P], tp[:], AF.Copy)
                    for m8 in range(LB):
                        ps = pd3_ps.tile([P, HID], FP, tag="pse")
                        for k2 in range(FB):
                            nc.tensor.matmul(
                                ps[:], pT[k2][:, m8 * P:(m8 + 1) * P], w2_sb[k2][:],
                                start=(k2 == 0), stop=(k2 == FB - 1))
                        et = pd3_sb.tile([P, HID], FP, tag="et")
                        nc.scalar.activation(et[:], ps[:], AF.Copy)
                        nc.sync.dma_start(emb_bounce[m8 * P:(m8 + 1) * P, :], et[:])
            ag(emb_bounce[:], emb_full[:])

            # ===== phase E: prototypes + cosine scores =====
            with (
                tc.tile_pool(name="pe_sb", bufs=1) as pe_sb,
                tc.tile_pool(name="pe_ps", bufs=1, space="PSUM") as pe_ps,
                tc.tile_pool(name="pe_sc", bufs=2) as pe_sc,
            ):
                idx_sb = pe_sb.tile([P, SB], mybir.dt.int32)
                nc.sync.dma_start(idx_sb[:], selidx[:])
                sel_sb = [pe_sb.tile([P, HID], FP, tag=f"sel{q}", name=f"sel{q}") for q in range(SB)]
                sc_q = [pe_sb.tile([P, 1], FP, tag=f"scq{q}", name=f"scq{q}") for q in range(SB)]
                oh_sb = [pe_sb.tile([P, NCLS], FP, tag=f"oh{q}", name=f"oh{q}") for q in range(SB)]
                for q in range(SB):
                    nc.gpsimd.indirect_dma_start(
                        out=sel_sb[q][:], out_offset=None,
                        in_=emb_full[:],
                        in_offset=bass.IndirectOffsetOnAxis(
                            ap=idx_sb[:, q:q + 1], axis=0))
                    nc.sync.dma_start(oh_sb[q][:], onehot[q * P:(q + 1) * P, :])
                    sq = pe_sc.tile([P, HID], FP, tag="sq")
                    ssq = pe_sc.tile([P, 1], FP, tag="ssq")
                    nc.scalar.activation(
                        sq[:], sel_sb[q][:], AF.Square, accum_out=ssq[:])
                    nrm = pe_sc.tile([P, 1], FP, tag="nrm2")
                    nc.scalar.activation(nrm[:], ssq[:], AF.Sqrt)
                    nc.vector.tensor_scalar_add(nrm[:], nrm[:], EPS)
                    nc.vector.tensor_scalar_mul(nrm[:], nrm[:], TEMP)
                    nc.vector.reciprocal(sc_q[q][:], nrm[:])
                sums_ps = [pe_ps.tile([P, NCLS], FP, tag=f"sums{b2}", name=f"sums{b2}")
                           for b2 in range(FB)]
                for q in range(SB):
                    for b2 in range(FB):
                        nc.tensor.matmul(
                            sums_ps[b2][:], sel_sb[q][:, b2 * P:(b2 + 1) * P],
                            oh_sb[q][:], start=(q == 0), stop=(q == SB - 1))
                for b2 in range(FB):
                    st = pe_sc.tile([P, NCLS], FP, tag="st")
                    nc.scalar.activation(st[:], sums_ps[b2][:], AF.Copy)
                    nc.sync.dma_start(sums_bounce[b2 * P:(b2 + 1) * P, :], st[:])
                nc.gpsimd.collective_compute(
                    "AllReduce", ALU.add, replica_groups=rg,
                    ins=[sums_bounce[:]], outs=[sums_red[:]])
                sums_sb = [pe_sb.tile([P, NCLS], FP, tag=f"smr{b2}", name=f"smr{b2}")
                           for b2 in range(FB)]
                ones_col = pe_sb.tile([P, 1], FP)
                nc.vector.memset(ones_col[:], 1.0)
                ones_row = pe_sb.tile([1, P], FP)
                nc.vector.memset(ones_row[:1, :], 1.0)
                nps = pe_ps.tile([1, NCLS], FP, tag="nps")
                for b2 in range(FB):
                    nc.sync.dma_start(sums_sb[b2][:], sums_red[b2 * P:(b2 + 1) * P, :])
                    sqs = pe_sc.tile([P, NCLS], FP, tag="sqs")
                    nc.scalar.activation(sqs[:], sums_sb[b2][:], AF.Square)
                    nc.tensor.matmul(nps[:1, :], ones_col[:, :1], sqs[:],
                                     start=(b2 == 0), stop=(b2 == FB - 1))
                nrmc = pe_sc.tile([1, NCLS], FP, tag="nrmc")
                nc.scalar.activation(nrmc[:1, :], nps[:1, :], AF.Sqrt)
                nc.vector.tensor_scalar_add(nrmc[:1, :], nrmc[:1, :], EPS)
                invc = pe_sc.tile([1, NCLS], FP, tag="invc")
                nc.vector.reciprocal(invc[:1, :], nrmc[:1, :])
                bcp = pe_ps.tile([P, NCLS], FP, tag="bcp")
                nc.tensor.matmul(bcp[:], ones_row[:1, :], invc[:1, :],
                                 start=True, stop=True)
                bc_sb = pe_sb.tile([P, NCLS], FP)
                nc.scalar.activation(bc_sb[:], bcp[:], AF.Copy)
                pnT = [pe_sb.tile([P, NCLS], FP, tag=f"pnT{b2}", name=f"pnT{b2}") for b2 in range(FB)]
                for b2 in range(FB):
                    nc.vector.tensor_mul(pnT[b2][:], sums_sb[b2][:], bc_sb[:])
                selT = [pe_sb.tile([P, SELC], FP, tag=f"selT{b2}", name=f"selT{b2}")
                        for b2 in range(FB)]
                for q in range(SB):
                    for b2 in range(FB):
                        tp = pe_ps.tile([P, P], FP, tag="tpe")
                        nc.tensor.transpose(
                            tp[:], sel_sb[q][:, b2 * P:(b2 + 1) * P], ident[:])
                        nc.scalar.activation(
                            selT[b2][:, q * P:(q + 1) * P], tp[:], AF.Copy)
                for q in range(SB):
                    ops = pe_ps.tile([P, NCLS], FP, tag="ops")
                    for b2 in range(FB):
                        nc.tensor.matmul(
                            ops[:], selT[b2][:, q * P:(q + 1) * P], pnT[b2][:],
                            start=(b2 == 0), stop=(b2 == FB - 1))
                    ot = pe_sc.tile([P, NCLS], FP, tag="ot")
                    nc.scalar.activation(ot[:], ops[:], AF.Copy, scale=sc_q[q][:, :1])
                    nc.sync.dma_start(out[q * P:(q + 1) * P, :], ot[:])

            if debug_outputs:
                nc.sync.dma_start(dbg_t[:], t_bounce[:])
                nc.sync.dma_start(dbg_hnT[:], hnT_bounce[:])
                nc.sync.dma_start(dbg_h1[:], h1_bounce[:])
                nc.sync.dma_start(dbg_emb[:], emb_bounce[:])

    nc.finalize()
    return nc


# ---------------------------------------------------------------------------
# host side
# ---------------------------------------------------------------------------

def host_preprocess(inputs, n=N, ncores=NCORES, nsel=NSEL):
    R = n // ncores
    selc = nsel // ncores
    x = np.ascontiguousarray(np.asarray(inputs["x"], dtype=np.float32))
    cw = np.asarray(inputs["combine_weight"], dtype=np.float32)
    alpha = float(np.asarray(inputs["alpha"], dtype=np.float32))
    prompt = np.asarray(inputs["prompt_spec"], dtype=np.float32)
    shared = np.asarray(inputs["shared_tok"], dtype=np.float32)
    baltok = np.asarray(inputs["balance_tok"], dtype=np.float32)
    w1 = np.ascontiguousarray(np.asarray(inputs["W1"], dtype=np.float32))
    w2 = np.ascontiguousarray(np.asarray(inputs["W2"], dtype=np.float32))
    edge_index = np.asarray(inputs["edge_index"])
    labels = np.asarray(inputs["labels"])
    node_idx = np.asarray(inputs["node_idx"])

    src = edge_index[0].astype(np.int64)
    dst = edge_index[1].astype(np.int64)
    deg = (np.bincount(dst, minlength=n) + 1).astype(np.float32)
    dinv = deg ** -0.5
    wn = (dinv[src] * dinv[dst]).astype(np.float32)
    # AnormT[src, dst] += wn  (transpose of reference's Anorm[dst, src] += wn)
    anormT = np.zeros((n, n), dtype=np.float32)
    np.add.at(anormT, (src, dst), wn)
    anormT[np.arange(n), np.arange(n)] += dinv * dinv
    if alpha > 1e-6:
        # device expects alpha-scaled adjacency (see build_program use_anorm)
        anormT *= alpha

    wcomb = (cw[0, 0] * prompt + cw[0, 1] * shared).astype(np.float32).reshape(1, -1)
    baltok2 = np.ascontiguousarray(baltok.reshape(1, -1))

    nsel_here = node_idx.shape[0]
    onehot_all = np.zeros((nsel_here, NCLS), dtype=np.float32)
    onehot_all[np.arange(nsel_here), labels] = 1.0

    in_maps = []
    for c in range(ncores):
        sel_slice = node_idx[c * selc:(c + 1) * selc].astype(np.int32)
        sb = selc // P
        in_maps.append({
            "x_l": x[c * R:(c + 1) * R, :],
            "wcomb": wcomb,
            "baltok": baltok2,
            "anormT": np.ascontiguousarray(anormT[:, c * R:(c + 1) * R]),
            "w1": w1,
            "w2": w2,
            "selidx": np.ascontiguousarray(sel_slice.reshape(sb, P).T),
            "onehot": np.ascontiguousarray(onehot_all[c * selc:(c + 1) * selc, :]),
        })
    return alpha, in_maps


_prog_cache = {}


def kernel(**inputs) -> np.ndarray:
    alpha, in_maps = host_preprocess(inputs)
    key = round(alpha, 9)
    if key not in _prog_cache:
        _prog_cache[key] = build_program(alpha)
    nc = _prog_cache[key]
    res = run_bass_kernel_spmd(nc, in_maps, list(range(NCORES)))
    return np.concatenate([res.results[c]["out"] for c in range(NCORES)], axis=0)


# revision 23
# speedup vs baseline: 1.3344x; 1.1670x over previous
"""Trainium2 Bass kernel for nn_Downstream_38439957299924 (gnn_message_passing).

Distributed over 8 NeuronCores: 1D node partition (1024 rows/core).

Pipeline (reformulated, validated vs reference at ~9e-7 rel err in numpy):
  fea   = elu(x * wcomb)                          wcomb = cw00*prompt + cw01*shared
  agg   = Anorm @ fea                             Anorm built dense on host from edges
  h     = concat(fea, agg) * balance_tok ; hn = h / (||h|| + eps)
  sims  = hn @ hn.T ; t_i = 17th largest of row i
  Wsym  = relu(sims * (sims >= min(t_i, t_j)))    == to_undirected(mean)+relu of ref
  A_tot = alpha*Anorm + (1-alpha)*Wsym
  h1    = relu((A_tot @ fea) @ W1) ; emb = (A_tot @ h1) @ W2
  out   = cos(emb[node_idx], class prototypes) / TEMP

Sharding: each core owns N/8 node rows. fea/hnT/h1/emb are all-gathered; the
N x N stages only materialize [128, *] tiles; per-row top-k via max8 +
match_replace candidate extraction; Wsym needs no transpose communication
because sims is symmetric and the mask only needs the threshold vector t.
"""
import numpy as np

import concourse.bacc as bacc
import concourse.bass as bass
import concourse.mybir as mybir
import concourse.tile as tile
from concourse.bass_utils import run_bass_kernel_spmd
from concourse.masks import make_identity

FP = mybir.dt.float32
HF = mybir.dt.float16
AF = mybir.ActivationFunctionType
ALU = mybir.AluOpType

NCORES = 8
N = 8192          # nodes
F = 256           # input feature dim
H2 = 512          # concat feature dim
HID = 256         # gnn hidden dim
NCLS = 10
NSEL = 4096
TEMP = 0.2
EPS = 1e-8
P = 128

NEGINF = -3.0e38


def build_program(alpha: float, n=N, ncores=NCORES, nsel=NSEL, mm_dtype=FP,
                  debug_outputs=False):
    """Emit the SPMD Bass program for one core (SPMD across ncores)."""
    R = n // ncores           # rows per core
    NB = n // P               # global node blocks of 128
    LB = R // P               # local node blocks
    KB = H2 // P              # feature k-blocks (4)
    FB = F // P               # 256-dim k-blocks (2)
    NCH = n // 512            # 512-wide j chunks globally
    NCH_L = R // 512          # 512-wide i chunks locally
    CH_PER_RANK = R // 512
    SELC = nsel // ncores     # selected nodes per core
    SB = SELC // P            # selected blocks per core
    QCH = (LB + 1) // 2       # psum tiles (2 x 256-wide outputs each)
    one_m_alpha = 1.0 - alpha
    # host ships alpha*AnormT when alpha is meaningful (lets phase C use a plain
    # tensor add on gpsimd); agg then un-scales at psum evict.
    use_anorm = alpha > 1e-6
    agg_scale = (1.0 / alpha) if use_anorm else 1.0
    assert R % 512 == 0 and SELC % P == 0

    nc = bacc.Bacc(None)

    # ---- per-core external inputs ----
    x_l = nc.declare_dram_parameter("x_l", [R, F], FP, isOutput=False)
    wcomb = nc.declare_dram_parameter("wcomb", [1, F], FP, isOutput=False)
    baltok = nc.declare_dram_parameter("baltok", [1, H2], FP, isOutput=False)
    anormT = nc.declare_dram_parameter("anormT", [n, R], FP, isOutput=False)
    anorm16 = nc.declare_dram_parameter("anorm16", [n, R], HF, isOutput=False)
    w1 = nc.declare_dram_parameter("w1", [F, HID], FP, isOutput=False)
    w2 = nc.declare_dram_parameter("w2", [HID, HID], FP, isOutput=False)
    selidx = nc.declare_dram_parameter("selidx", [P, SB], mybir.dt.int32, isOutput=False)
    onehot = nc.declare_dram_parameter("onehot", [SELC, NCLS], FP, isOutput=False)
    out = nc.declare_dram_parameter("out", [SELC, NCLS], FP, isOutput=True)
    if debug_outputs:
        dbg_t = nc.declare_dram_parameter("dbg_t", [R, 1], FP, isOutput=True)
        dbg_hnT = nc.declare_dram_parameter("dbg_hnT", [H2, R], FP, isOutput=True)
        dbg_h1 = nc.declare_dram_parameter("dbg_h1", [R, HID], HF, isOutput=True)
        dbg_emb = nc.declare_dram_parameter("dbg_emb", [R, HID], FP, isOutput=True)
        dbg_agg = nc.declare_dram_parameter("dbg_agg", [F, R], FP, isOutput=True)

    # ---- internal DRAM ----
    fea_bounce = nc.dram_tensor("fea_bounce", [R, F], FP)
    fea_full = nc.dram_tensor("fea_full", [n, F], FP, addr_space="Shared")
    hnT_bounce = nc.dram_tensor("hnT_bounce", [H2, R], FP)
    hnT_all = nc.dram_tensor("hnT_all", [ncores * H2, R], FP, addr_space="Shared")
    t_bounce = nc.dram_tensor("t_bounce", [R, 1], FP)
    t_full = nc.dram_tensor("t_full", [n, 1], FP, addr_space="Shared")
    atotT_dram = nc.dram_tensor("atotT_dram", [n, R], HF)
    simsT_dram = nc.dram_tensor("simsT_dram", [n, R], FP)
    fea16_bounce = nc.dram_tensor("fea16_bounce", [R, F], HF)
    fea16_full = nc.dram_tensor("fea16_full", [n, F], HF, addr_space="Shared")
    h1_bounce = nc.dram_tensor("h1_bounce", [R, HID], HF)
    h1_full = nc.dram_tensor("h1_full", [n, HID], HF, addr_space="Shared")
    emb_bounce = nc.dram_tensor("emb_bounce", [R, HID], FP)
    emb_full = nc.dram_tensor("emb_full", [n, HID], FP, addr_space="Shared")
    sums_bounce = nc.dram_tensor("sums_bounce", [HID, NCLS], FP)
    sums_red = nc.dram_tensor("sums_red", [HID, NCLS], FP, addr_space="Shared")

    rg = [list(range(ncores))]

    def dma_cast(out_ap, in_ap):
        """DMA that may convert f32 -> f32r; casting DMAs must come from gpsimd."""
        eng = nc.gpsimd if mm_dtype != FP else nc.sync
        eng.dma_start(out_ap, in_ap)

    def ag(in_ap, out_ap):
        nc.gpsimd.collective_compute(
            "AllGather", ALU.bypass, replica_groups=rg, ins=[in_ap], outs=[out_ap])

    def hnT_all_tile(jblk, kk):
        """[128 k, 128 j] slice of the gathered hnT for global node block jblk."""
        r, jb = jblk // LB, jblk % LB
        base = r * H2 + kk * P
        return hnT_all[base:base + P, jb * P:(jb + 1) * P]

    with tile.TileContext(nc) as tc:
        with (
            tc.tile_pool(name="const", bufs=1) as const,
            tc.tile_pool(name="live", bufs=1) as live,
        ):
            ident = const.tile([P, P], FP)
            make_identity(nc, ident)
            wcomb_b = const.tile([P, F], FP)
            nc.sync.dma_start(wcomb_b[:], wcomb[:1, :].to_broadcast([P, F]))
            baltok_b = const.tile([P, H2], FP)
            nc.sync.dma_start(baltok_b[:], baltok[:1, :].to_broadcast([P, H2]))
            hnT_l = [live.tile([P, R], FP, tag=f"hnT{k}", name=f"hnT{k}") for k in range(KB)]

            # ===== phase 0: fea_l = elu(x_l * wcomb); all-gather fea =====
            with tc.tile_pool(name="p0", bufs=3) as p0:
                for b in range(LB):
                    xt = p0.tile([P, F], FP, tag="xt")
                    nc.sync.dma_start(xt[:], x_l[b * P:(b + 1) * P, :])
                    z = p0.tile([P, F], FP, tag="z")
                    nc.vector.tensor_mul(z[:], xt[:], wcomb_b[:, :F])
                    r = p0.tile([P, F], FP, tag="r")
                    nc.scalar.activation(r[:], z[:], AF.Relu)
                    m0 = p0.tile([P, F], FP, tag="m0")
                    nc.vector.tensor_scalar_min(m0[:], z[:], 0.0)
                    e = p0.tile([P, F], FP, tag="e")
                    nc.scalar.activation(e[:], m0[:], AF.Exp)
                    ft = p0.tile([P, F], FP, tag="ft")
                    nc.vector.scalar_tensor_tensor(
                        ft[:], e[:], -1.0, r[:], op0=ALU.add, op1=ALU.add)
                    nc.sync.dma_start(fea_bounce[b * P:(b + 1) * P, :], ft[:])
                    f16 = p0.tile([P, F], HF, tag="f16")
                    nc.scalar.activation(f16[:], ft[:], AF.Copy)
                    nc.sync.dma_start(fea16_bounce[b * P:(b + 1) * P, :], f16[:])
            ag(fea_bounce[:], fea_full[:])
            ag(fea16_bounce[:], fea16_full[:])

            # ===== phase A: aggT = fea.T @ anormT; then h, hn, hnT =====
            with tc.tile_pool(name="pa_sb", bufs=1) as pa_sb:
                aggT_sb = [pa_sb.tile([P, R], FP, tag=f"aggT{mf}", name=f"aggT{mf}") for mf in range(FB)]
                with (
                    tc.tile_pool(name="pa_in", bufs=3) as pa_in,
                    tc.tile_pool(name="pa_ps", bufs=1, space="PSUM") as pa_ps,
                ):
                    aggT_ps = [[pa_ps.tile([P, 512], FP, tag=f"aggps{mf}_{c2}", name=f"aggps{mf}_{c2}")
                                for c2 in range(NCH_L)] for mf in range(FB)]
                    for kj in range(NB):
                        fk = pa_in.tile([P, F], mm_dtype, tag="fk")
                        dma_cast(fk[:], fea_full[kj * P:(kj + 1) * P, :])
                        an = pa_in.tile([P, R], mm_dtype, tag="an")
                        dma_cast(an[:], anormT[kj * P:(kj + 1) * P, :])
                        for mf in range(FB):
                            for c2 in range(NCH_L):
                                nc.tensor.matmul(
                                    aggT_ps[mf][c2][:],
                                    fk[:, mf * P:(mf + 1) * P],
                                    an[:, c2 * 512:(c2 + 1) * 512],
                                    start=(kj == 0), stop=(kj == NB - 1))
                    for mf in range(FB):
                        for c2 in range(NCH_L):
                            nc.scalar.activation(
                                aggT_sb[mf][:, c2 * 512:(c2 + 1) * 512],
                                aggT_ps[mf][c2][:], AF.Copy, scale=agg_scale)
                    if debug_outputs:
                        for mf in range(FB):
                            nc.sync.dma_start(
                                dbg_agg[mf * P:(mf + 1) * P, :], aggT_sb[mf][:])

                with (
                    tc.tile_pool(name="ph_ps", bufs=2, space="PSUM") as ph_ps,
                    tc.tile_pool(name="ph_sb", bufs=2) as ph_sb,
                ):
                    for b in range(LB):
                        h = ph_sb.tile([P, H2], FP, tag="h")
                        ft = ph_sb.tile([P, F], FP, tag="ftl")
                        nc.sync.dma_start(ft[:], fea_bounce[b * P:(b + 1) * P, :])
                        nc.vector.tensor_mul(h[:, :F], ft[:], baltok_b[:, :F])
                        for mf in range(FB):
                            tp = ph_ps.tile([P, P], FP, tag="tp")
                            nc.tensor.transpose(
                                tp[:], aggT_sb[mf][:, b * P:(b + 1) * P], ident[:])
                            nc.scalar.activation(
                                h[:, F + mf * P:F + (mf + 1) * P], tp[:], AF.Copy)
                        nc.vector.tensor_mul(h[:, F:], h[:, F:], baltok_b[:, F:])
                        sq = ph_sb.tile([P, H2], FP, tag="sq")
                        ssq = ph_sb.tile([P, 1], FP, tag="ssq")
                        nc.scalar.activation(sq[:], h[:], AF.Square, accum_out=ssq[:])
                        nrm = ph_sb.tile([P, 1], FP, tag="nrm")
                        nc.scalar.activation(nrm[:], ssq[:], AF.Sqrt)
                        nc.vector.tensor_scalar_add(nrm[:], nrm[:], EPS)
                        inv = ph_sb.tile([P, 1], FP, tag="inv")
                        nc.vector.reciprocal(inv[:], nrm[:])
                        hn = ph_sb.tile([P, H2], FP, tag="hn")
                        nc.vector.tensor_scalar(hn[:], h[:], inv[:, :1], None, ALU.mult)
                        for kk in range(KB):
                            tp2 = ph_ps.tile([P, P], FP, tag="tp2")
                            nc.tensor.transpose(
                                tp2[:], hn[:, kk * P:(kk + 1) * P], ident[:])
                            nc.scalar.activation(
                                hnT_l[kk][:, b * P:(b + 1) * P], tp2[:], AF.Copy)
                    for kk in range(KB):
                        nc.sync.dma_start(
                            hnT_bounce[kk * P:(kk + 1) * P, :], hnT_l[kk][:])
            ag(hnT_bounce[:], hnT_all[:])

            # ===== phase C1: simsT tiles (single fp32 pass), spill to DRAM,
            #       extract per-row top-k candidates from PE-transposed tiles =====
            with tc.tile_pool(name="pb_cand", bufs=1) as pb_cand:
                cand = [pb_cand.tile([P, 8 * NB], FP, tag=f"cand{m8}", name=f"cand{m8}")
                        for m8 in range(LB)]
                with (
                    tc.tile_pool(name="pc1_in", bufs=3) as pc1_in,
                    tc.tile_pool(name="pc1_ps", bufs=2, space="PSUM") as pc1_ps,
                    tc.tile_pool(name="pc1_tp", bufs=2, space="PSUM") as pc1_tp,
                    tc.tile_pool(name="pc1_sb", bufs=3) as pc1_sb,
                ):
                    for mj in range(NB):
                        lhs = []
                        for kk in range(KB):
                            lt = pc1_in.tile([P, P], FP, tag=f"lhs{kk}")
                            nc.sync.dma_start(lt[:], hnT_all_tile(mj, kk))
                            lhs.append(lt)
                        for hh in range(NCH_L):
                            ps = pc1_ps.tile([P, 512], FP, tag="ps")
                            for kk in range(KB):
                                nc.tensor.matmul(
                                    ps[:], lhs[kk][:],
                                    hnT_l[kk][:, hh * 512:(hh + 1) * 512],
                                    start=(kk == 0), stop=(kk == KB - 1))
                            st = pc1_sb.tile([P, 512], FP, tag="st")
                            nc.scalar.activation(st[:], ps[:], AF.Copy)
                            nc.sync.dma_start(
                                simsT_dram[mj * P:(mj + 1) * P,
                                           hh * 512:(hh + 1) * 512], st[:])
                            for sub in range(4):
                                ib = hh * 4 + sub
                                tp = pc1_tp.tile([P, P], FP, tag="tp")
                                nc.tensor.transpose(
                                    tp[:], st[:, sub * P:(sub + 1) * P], ident[:])
                                tr = pc1_sb.tile([P, P], FP, tag="tr")
                                nc.scalar.activation(tr[:], tp[:], AF.Copy)
                                nc.vector.max(
                                    cand[ib][:, mj * 8:mj * 8 + 8], tr[:])
                # merge candidates -> t (17th largest per local row)
                with tc.tile_pool(name="pbm_sb", bufs=2) as pbm_sb:
                    for m8 in range(LB):
                        t8a = pbm_sb.tile([P, 8], FP, tag="t8a")
                        nc.vector.max(t8a[:], cand[m8][:])
                        nc.vector.match_replace(
                            cand[m8][:], t8a[:], cand[m8][:], NEGINF)
                        t8b = pbm_sb.tile([P, 8], FP, tag="t8b")
                        nc.vector.max(t8b[:], cand[m8][:])
                        nc.vector.match_replace(
                            cand[m8][:], t8b[:], cand[m8][:], NEGINF)
                        t8c = pbm_sb.tile([P, 8], FP, tag="t8c")
                        nc.vector.max(t8c[:], cand[m8][:])
                        nc.sync.dma_start(
                            t_bounce[m8 * P:(m8 + 1) * P, :], t8c[:, :1])
            ag(t_bounce[:], t_full[:])

            # ===== phase C2: A_totT = alpha*Anorm + (1-alpha)*relu(masked sims)
            #       in fp16, fused Q = A_tot @ fea =====
            with tc.tile_pool(name="pcd_sb", bufs=1) as pcd_sb:
                q_sb = [pcd_sb.tile([P, F], FP, tag=f"qsb{m8}", name=f"qsb{m8}")
                        for m8 in range(LB)]
                with (
                    tc.tile_pool(name="pc_tib", bufs=1) as pc_tib,
                    tc.tile_pool(name="pc_in", bufs=3) as pc_in,
                    tc.tile_pool(name="pc_qps", bufs=1, space="PSUM") as pc_qps,
                    tc.tile_pool(name="pc_sb", bufs=3) as pc_sb,
                ):
                    t_i_b = pc_tib.tile([P, R], FP)
                    nc.sync.dma_start(
                        t_i_b[:],
                        t_bounce.rearrange("a b -> b a")[:1, :].to_broadcast([P, R]))
                    # t_full loaded once as [128, NB]: column mj = t for node block mj
                    tf_sb = pc_tib.tile([P, NB], FP)
                    nc.sync.dma_start(
                        tf_sb[:], t_full.rearrange("(m p) one -> p (m one)", p=P))
                    qps = [pc_qps.tile([P, 512], FP, tag=f"q{q}", name=f"q{q}")
                           for q in range(QCH)]
                    for mj in range(NB):
                        st = pc_in.tile([P, R], FP, tag="st2")
                        nc.sync.dma_start(st[:], simsT_dram[mj * P:(mj + 1) * P, :])
                        feq = pc_in.tile([P, F], HF, tag="feq")
                        nc.sync.dma_start(feq[:], fea16_full[mj * P:(mj + 1) * P, :])
                        if use_anorm:
                            an = pc_in.tile([P, R], HF, tag="an")
                            nc.sync.dma_start(
                                an[:], anorm16[mj * P:(mj + 1) * P, :])
                        tmin = pc_sb.tile([P, R], FP, tag="tmin")
                        nc.vector.tensor_scalar(
                            tmin[:], t_i_b[:], tf_sb[:, mj:mj + 1], None, ALU.min)
                        ge = pc_sb.tile([P, R], FP, tag="ge")
                        nc.vector.tensor_tensor(ge[:], st[:], tmin[:], ALU.is_ge)
                        w = pc_sb.tile([P, R], FP, tag="w")
                        nc.vector.tensor_mul(w[:], st[:], ge[:])
                        ws = pc_sb.tile([P, R], HF, tag="ws")
                        nc.scalar.activation(ws[:], w[:], AF.Relu, scale=one_m_alpha)
                        at = pc_sb.tile([P, R], HF, tag="at")
                        if use_anorm:
                            nc.gpsimd.tensor_tensor(at[:], an[:], ws[:], ALU.add)
                        else:
                            nc.vector.tensor_copy(at[:], ws[:])
                        nc.sync.dma_start(
                            atotT_dram[mj * P:(mj + 1) * P, :], at[:])
                        for m8 in range(LB):
                            nc.tensor.matmul(
                                qps[m8 // 2][:, (m8 % 2) * F:(m8 % 2 + 1) * F],
                                at[:, m8 * P:(m8 + 1) * P], feq[:],
                                start=(mj == 0 and m8 % 2 == 0),
                                stop=(mj == NB - 1 and m8 % 2 == 1))
                    for m8 in range(LB):
                        nc.scalar.activation(
                            q_sb[m8][:],
                            qps[m8 // 2][:, (m8 % 2) * F:(m8 % 2 + 1) * F], AF.Copy)

                # ===== phase D1: h1 = relu(Q @ W1); all-gather h1 =====
                with (
                    tc.tile_pool(name="pd1_ps", bufs=2, space="PSUM") as pd1_ps,
                    tc.tile_pool(name="pd1_sb", bufs=2) as pd1_sb,
                    tc.tile_pool(name="pd1_w", bufs=1) as pd1_w,
                ):
                    w1_sb = [pd1_w.tile([P, HID], FP, tag=f"w1_{k2}", name=f"w1_{k2}")
                             for k2 in range(FB)]
                    for k2 in range(FB):
                        nc.sync.dma_start(w1_sb[k2][:], w1[k2 * P:(k2 + 1) * P, :])
                    qT = [pd1_w.tile([P, R], FP, tag=f"qT{k2}", name=f"qT{k2}")
                          for k2 in range(FB)]
                    for m8 in range(LB):
                        for k2 in range(FB):
                            tp = pd1_ps.tile([P, P], FP, tag="tp")
                            nc.tensor.transpose(
                                tp[:], q_sb[m8][:, k2 * P:(k2 + 1) * P], ident[:])
                            nc.scalar.activation(
                                qT[k2][:, m8 * P:(m8 + 1) * P], tp[:], AF.Copy)
                    for m8 in range(LB):
                        ps = pd1_ps.tile([P, HID], FP, tag="psh")
                        for k2 in range(FB):
                            nc.tensor.matmul(
                                ps[:], qT[k2][:, m8 * P:(m8 + 1) * P], w1_sb[k2][:],
                                start=(k2 == 0), stop=(k2 == FB - 1))
                        h1t = pd1_sb.tile([P, HID], HF, tag="h1t")
                        nc.scalar.activation(h1t[:], ps[:], AF.Relu)
                        nc.sync.dma_start(h1_bounce[m8 * P:(m8 + 1) * P, :], h1t[:])
            ag(h1_bounce[:], h1_full[:])

            # ===== phase D2: P = A_tot @ h1_full =====
            with tc.tile_pool(name="pdd_sb", bufs=1) as pdd_sb:
                p_sb = [pdd_sb.tile([P, HID], FP, tag=f"psb{m8}", name=f"psb{m8}") for m8 in range(LB)]
                with (
                    tc.tile_pool(name="pd2_in", bufs=3) as pd2_in,
                    tc.tile_pool(name="pd2_ps", bufs=1, space="PSUM") as pd2_ps,
                ):
                    pps = [pd2_ps.tile([P, 512], FP, tag=f"p{q}", name=f"pp{q}") for q in range(QCH)]
                    for kj in range(NB):
                        at = pd2_in.tile([P, R], HF, tag="at")
                        nc.sync.dma_start(at[:], atotT_dram[kj * P:(kj + 1) * P, :])
                        h1k = pd2_in.tile([P, HID], HF, tag="h1k")
                        nc.sync.dma_start(h1k[:], h1_full[kj * P:(kj + 1) * P, :])
                        for m8 in range(LB):
                            nc.tensor.matmul(
                                pps[m8 // 2][:, (m8 % 2) * HID:(m8 % 2 + 1) * HID],
                                at[:, m8 * P:(m8 + 1) * P], h1k[:],
                                start=(kj == 0 and m8 % 2 == 0),
                                stop=(kj == NB - 1 and m8 % 2 == 1))
                    for m8 in range(LB):
                        nc.scalar.activation(
                            p_sb[m8][:],
                            pps[m8 // 2][:, (m8 % 2) * HID:(m8 % 2 + 1) * HID],
                            AF.Copy)

                # ===== phase D3: emb = P @ W2; all-gather emb =====
                with (
                    tc.tile_pool(name="pd3_ps", bufs=2, space="PSUM") as pd3_ps,
                    tc.tile_pool(name="pd3_w", bufs=1) as pd3_w,
                    tc.tile_pool(name="pd3_sb", bufs=2) as pd3_sb,
                ):
                    w2_sb = [pd3_w.tile([P, HID], FP, tag=f"w2_{k2}", name=f"w2_{k2}")
                             for k2 in range(FB)]
                    for k2 in range(FB):
                        nc.sync.dma_start(w2_sb[k2][:], w2[k2 * P:(k2 + 1) * P, :])
                    pT = [pd3_w.tile([P, R], FP, tag=f"pT{k2}", name=f"pT{k2}")
                          for k2 in range(FB)]
                    for m8 in range(LB):
                        for k2 in range(FB):
                            tp = pd3_ps.tile([P, P], FP, tag="tp")
                            nc.tensor.transpose(
                                tp[:], p_sb[m8][:, k2 * P:(k2 + 1) * P], ident[:])
                            nc.scalar.activation(
                                pT[k2][:, m8 * P:(m8 + 1) * P], tp[:], AF.Copy)
                    for m8 in range(LB):
                        ps = pd3_ps.tile([P, HID], FP, tag="pse")
                        for k2 in range(FB):
                            nc.tensor.matmul(
                                ps[:], pT[k2][:, m8 * P:(m8 + 1) * P], w2_sb[k2][:],
                                start=(k2 == 0), stop=(k2 == FB - 1))
                        et = pd3_sb.tile([P, HID], FP, tag="et")
                        nc.scalar.activation(et[:], ps[:], AF.Copy)
                        nc.sync.dma_start(emb_bounce[m8 * P:(m8 + 1) * P, :], et[:])
            ag(emb_bounce[:], emb_full[:])

            # ===== phase E: prototypes + cosine scores =====
            with (
                tc.tile_pool(name="pe_sb", bufs=1) as pe_sb,
                tc.tile_pool(name="pe_ps", bufs=1, space="PSUM") as pe_ps,
                tc.tile_pool(name="pe_sc", bufs=2) as pe_sc,
            ):
                idx_sb = pe_sb.tile([P, SB], mybir.dt.int32)
                nc.sync.dma_start(idx_sb[:], selidx[:])
                sel_sb = [pe_sb.tile([P, HID], FP, tag=f"sel{q}", name=f"sel{q}") for q in range(SB)]
                sc_q = [pe_sb.tile([P, 1], FP, tag=f"scq{q}", name=f"scq{q}") for q in range(SB)]
                oh_sb = [pe_sb.tile([P, NCLS], FP, tag=f"oh{q}", name=f"oh{q}") for q in range(SB)]
                for q in range(SB):
                    nc.gpsimd.indirect_dma_start(
                        out=sel_sb[q][:], out_offset=None,
                        in_=emb_full[:],
                        in_offset=bass.IndirectOffsetOnAxis(
                            ap=idx_sb[:, q:q + 1], axis=0))
                    nc.sync.dma_start(oh_sb[q][:], onehot[q * P:(q + 1) * P, :])
                    sq = pe_sc.tile([P, HID], FP, tag="sq")
                    ssq = pe_sc.tile([P, 1], FP, tag="ssq")
                    nc.scalar.activation(
                        sq[:], sel_sb[q][:], AF.Square, accum_out=ssq[:])
                    nrm = pe_sc.tile([P, 1], FP, tag="nrm2")
                    nc.scalar.activation(nrm[:], ssq[:], AF.Sqrt)
                    nc.vector.tensor_scalar_add(nrm[:], nrm[:], EPS)
                    nc.vector.tensor_scalar_mul(nrm[:], nrm[:], TEMP)
                    nc.vector.reciprocal(sc_q[q][:], nrm[:])
                sums_ps = [pe_ps.tile([P, NCLS], FP, tag=f"sums{b2}", name=f"sums{b2}")
                           for b2 in range(FB)]
                for q in range(SB):
                    for b2 in range(FB):
                        nc.tensor.matmul(
                            sums_ps[b2][:], sel_sb[q][:, b2 * P:(b2 + 1) * P],
                            oh_sb[q][:], start=(q == 0), stop=(q == SB - 1))
                for b2 in range(FB):
                    st = pe_sc.tile([P, NCLS], FP, tag="st")
                    nc.scalar.activation(st[:], sums_ps[b2][:], AF.Copy)
                    nc.sync.dma_start(sums_bounce[b2 * P:(b2 + 1) * P, :], st[:])
                nc.gpsimd.collective_compute(
                    "AllReduce", ALU.add, replica_groups=rg,
                    ins=[sums_bounce[:]], outs=[sums_red[:]])
                sums_sb = [pe_sb.tile([P, NCLS], FP, tag=f"smr{b2}", name=f"smr{b2}")
                           for b2 in range(FB)]
                ones_col = pe_sb.tile([P, 1], FP)
                nc.vector.memset(ones_col[:], 1.0)
                ones_row = pe_sb.tile([1, P], FP)
                nc.vector.memset(ones_row[:1, :], 1.0)
                nps = pe_ps.tile([1, NCLS], FP, tag="nps")
                for b2 in range(FB):
                    nc.sync.dma_start(sums_sb[b2][:], sums_red[b2 * P:(b2 + 1) * P, :])
                    sqs = pe_sc.tile([P, NCLS], FP, tag="sqs")
                    nc.scalar.activation(sqs[:], sums_sb[b2][:], AF.Square)
                    nc.tensor.matmul(nps[:1, :], ones_col[:, :1], sqs[:],
                                     start=(b2 == 0), stop=(b2 == FB - 1))
                nrmc = pe_sc.tile([1, NCLS], FP, tag="nrmc")
                nc.scalar.activation(nrmc[:1, :], nps[:1, :], AF.Sqrt)
                nc.vector.tensor_scalar_add(nrmc[:1, :], nrmc[:1, :], EPS)
                invc = pe_sc.tile([1, NCLS], FP, tag="invc")
                nc.vector.reciprocal(invc[:1, :], nrmc[:1, :])
                bcp = pe_ps.tile([P, NCLS], FP, tag="bcp")
                nc.tensor.matmul(bcp[:], ones_row[:1, :], invc[:1, :],
                                 start=True, stop=True)
                bc_sb = pe_sb.tile([P, NCLS], FP)
                nc.scalar.activation(bc_sb[:], bcp[:], AF.Copy)
                pnT = [pe_sb.tile([P, NCLS], FP, tag=f"pnT{b2}", name=f"pnT{b2}") for b2 in range(FB)]
                for b2 in range(FB):
                    nc.vector.tensor_mul(pnT[b2][:], sums_sb[b2][:], bc_sb[:])
                selT = [pe_sb.tile([P, SELC], FP, tag=f"selT{b2}", name=f"selT{b2}")
                        for b2 in range(FB)]
                for q in range(SB):
                    for b2 in range(FB):
                        tp = pe_ps.tile([P, P], FP, tag="tpe")
                        nc.tensor.transpose(
                            tp[:], sel_sb[q][:, b2 * P:(b2 + 1) * P], ident[:])
                        nc.scalar.activation(
                            selT[b2][:, q * P:(q + 1) * P], tp[:], AF.Copy)
                for q in range(SB):
                    ops = pe_ps.tile([P, NCLS], FP, tag="ops")
                    for b2 in range(FB):
                        nc.tensor.matmul(
                            ops[:], selT[b2][:, q * P:(q + 1) * P], pnT[b2][:],
                            start=(b2 == 0), stop=(b2 == FB - 1))
                    ot = pe_sc.tile([P, NCLS], FP, tag="ot")
                    nc.scalar.activation(ot[:], ops[:], AF.Copy, scale=sc_q[q][:, :1])
                    nc.sync.dma_start(out[q * P:(q + 1) * P, :], ot[:])

            if debug_outputs:
                nc.sync.dma_start(dbg_t[:], t_bounce[:])
                nc.sync.dma_start(dbg_hnT[:], hnT_bounce[:])
                nc.sync.dma_start(dbg_h1[:], h1_bounce[:])
                nc.sync.dma_start(dbg_emb[:], emb_bounce[:])

    nc.finalize()
    return nc


# ---------------------------------------------------------------------------
# host side
# ---------------------------------------------------------------------------

def host_preprocess(inputs, n=N, ncores=NCORES, nsel=NSEL):
    R = n // ncores
    selc = nsel // ncores
    x = np.ascontiguousarray(np.asarray(inputs["x"], dtype=np.float32))
    cw = np.asarray(inputs["combine_weight"], dtype=np.float32)
    alpha = float(np.asarray(inputs["alpha"], dtype=np.float32))
    prompt = np.asarray(inputs["prompt_spec"], dtype=np.float32)
    shared = np.asarray(inputs["shared_tok"], dtype=np.float32)
    baltok = np.asarray(inputs["balance_tok"], dtype=np.float32)
    w1 = np.ascontiguousarray(np.asarray(inputs["W1"], dtype=np.float32))
    w2 = np.ascontiguousarray(np.asarray(inputs["W2"], dtype=np.float32))
    edge_index = np.asarray(inputs["edge_index"])
    labels = np.asarray(inputs["labels"])
    node_idx = np.asarray(inputs["node_idx"])

    src = edge_index[0].astype(np.int64)
    dst = edge_index[1].astype(np.int64)
    deg = (np.bincount(dst, minlength=n) + 1).astype(np.float32)
    dinv = deg ** -0.5
    wn = (dinv[src] * dinv[dst]).astype(np.float32)
    # AnormT[src, dst] += wn  (transpose of reference's Anorm[dst, src] += wn)
    anormT = np.zeros((n, n), dtype=np.float32)
    np.add.at(anormT, (src, dst), wn)
    anormT[np.arange(n), np.arange(n)] += dinv * dinv
    if alpha > 1e-6:
        # device expects alpha-scaled adjacency (see build_program use_anorm)
        anormT *= alpha

    wcomb = (cw[0, 0] * prompt + cw[0, 1] * shared).astype(np.float32).reshape(1, -1)
    baltok2 = np.ascontiguousarray(baltok.reshape(1, -1))

    nsel_here = node_idx.shape[0]
    onehot_all = np.zeros((nsel_here, NCLS), dtype=np.float32)
    onehot_all[np.arange(nsel_here), labels] = 1.0

    in_maps = []
    for c in range(ncores):
        sel_slice = node_idx[c * selc:(c + 1) * selc].astype(np.int32)
        sb = selc // P
        an_slice = np.ascontiguousarray(anormT[:, c * R:(c + 1) * R])
        in_maps.append({
            "x_l": x[c * R:(c + 1) * R, :],
            "wcomb": wcomb,
            "baltok": baltok2,
            "anormT": an_slice,
            "anorm16": an_slice.astype(np.float16),
            "w1": w1,
            "w2": w2,
            "selidx": np.ascontiguousarray(sel_slice.reshape(sb, P).T),
            "onehot": np.ascontiguousarray(onehot_all[c * selc:(c + 1) * selc, :]),
        })
    return alpha, in_maps


_prog_cache = {}


def kernel(**inputs) -> np.ndarray:
    alpha, in_maps = host_preprocess(inputs)
    key = round(alpha, 9)
    if key not in _prog_cache:
        _prog_cache[key] = build_program(alpha)
    nc = _prog_cache[key]
    res = run_bass_kernel_spmd(nc, in_maps, list(range(NCORES)))
    return np.concatenate([res.results[c]["out"] for c in range(NCORES)], axis=0)
